# revision 37
# baseline (speedup 1.0000x reference)
"""Trainium2 Bass kernel for AdaptiveRouter MoE routing.

reference:
  logits = hidden @ W^T + adaptive_bias + ema/||ema||
  scores = softmax(logits); topk_w, topk_i = top_k(scores, 8); topk_w /= sum
  aux = E * sum(tokens_per_expert * mean_prob)

Sharding: data-parallel over tokens across 8 cores. Router weight/bias
replicated. Aux-loss partial sums (sum of scores per expert, top-8 counts
per expert) are computed per core and combined on the host during gather.

Numerics: fp32 matmul on TRN2 runs in LOW_HIGH 2-pass mode with
self-loading 4-byte weight loads (slow). Instead the host splits X into
bf16 hi/lo planes (x = x_hi + x_lo) and packs W^T as [W_hi | W_lo]
(128 bf16 columns, FWL-eligible). Streaming x_hi then x_lo against the
packed weights accumulates all four product terms across a (128, 512)
PSUM tile: rows 0-63 = W_hi*(x_hi+x_lo), rows 64-127 = W_lo*(x_hi+x_lo).
The halves are summed after the PE transpose as a free-dim add.
Max logit error ~1.7e-5 -> ~1 of 131072 top-8 indices flip vs fp32.

Device kernel (per core, T=2048 tokens, H=4096, E=64):
  - 2 halves x 1024 tokens; per half, 32 k-chunks stream into 2 PSUM blocks
  - bias ([bias;0] over the 128 packed rows) added during PSUM->SBUF evac
  - PE transpose -> (128 tok, 128) tiles; hi+lo halves summed -> logits
  - DVE max/max_index -> top-8 values + indices per token (descending)
  - ACT exp (logits are O(6); no max subtraction needed for fp32 range)
  - topk weights = exp(top8) / sum(exp(top8))
  - stats via bf16 PE "ones-matmul" over tokens: sum_t exp(l)/denom, mask
"""

import os

import numpy as np

import concourse.bass as bass
import concourse.mybir as mybir
from concourse import bacc
from concourse import tile
from concourse.bass_utils import run_bass_kernel_spmd

F32 = mybir.dt.float32
BF16 = mybir.dt.bfloat16
F16 = mybir.dt.float16
F8 = mybir.dt.float8e4
U32 = mybir.dt.uint32
S_LO = 1024.0  # host scale on the fp8 residual plane
S_W = 64.0  # host scale on the W-residual / fp8-W planes

N_TOKENS = 16384
HIDDEN = 4096
N_EXPERTS = 64
TOP_K = 8
N_CORES = 8
T_CORE = N_TOKENS // N_CORES  # 2048
K_CHUNKS = HIDDEN // 128  # 32
BLK = 512  # tokens per PSUM block
HALF = 1024  # tokens per outer iteration (2 PSUM blocks)
N_HALF = T_CORE // HALF  # 2
SUB = BLK // 128  # 4 sub-tiles of 128 tokens per block

_CACHED_NC = None
LAST_RESULTS = None


def _install_ntff_hook():
    """concourse's traced axon path imports antenv.axon_hooks, which this
    image lacks. Recreate it in sys.modules and register the ctypes-based
    NTFF profile hook from trn_agent_boot."""
    import sys
    import types

    if "antenv.axon_hooks" in sys.modules:
        return True
    try:
        import antenv
        from trn_agent_boot.trn_boot import _ntff_profile_via_ctypes

        mod = types.ModuleType("antenv.axon_hooks")
        mod._hook = _ntff_profile_via_ctypes("/opt/axon/libaxon_pjrt.so")

        def set_axon_ntff_profile_hook(h):
            mod._hook = h

        def get_axon_ntff_profile_hook():
            return mod._hook

        mod.set_axon_ntff_profile_hook = set_axon_ntff_profile_hook
        mod.get_axon_ntff_profile_hook = get_axon_ntff_profile_hook
        sys.modules["antenv.axon_hooks"] = mod
        antenv.axon_hooks = mod
        return True
    except Exception:
        return False


def _topk_block(nc, pools, logits_blk, nsub):
    """Top-8 + softmax + stats inputs over nsub 128-token sub-tiles.

    logits_blk: (128, nsub*64) f32 SBUF tile of logits in (token, expert)
    layout. Produces output tiles plus fused stats operands:
    em = [exp(l) | top8-mask] (128, nsub*128) bf16, ro = [recip | 1] pairs.
    """
    SUB = nsub
    tok_pool, out_pool = pools
    top8_blk = tok_pool.tile([128, SUB * TOP_K], F32)
    idx_blk = out_pool.tile([128, SUB * TOP_K], U32)
    em_blk = tok_pool.tile([128, SUB * 128], BF16)
    em3 = em_blk[:].rearrange("p (t c) -> p t c", t=SUB)
    for t in range(SUB):
        lsl = logits_blk[:, t * N_EXPERTS : (t + 1) * N_EXPERTS]
        t8 = top8_blk[:, t * TOP_K : (t + 1) * TOP_K]
        nc.vector.max(out=t8, in_=lsl)
        nc.vector.max_index(
            out=idx_blk[:, t * TOP_K : (t + 1) * TOP_K], in_max=t8, in_values=lsl
        )
    logits3 = logits_blk[:].rearrange("p (t e) -> p t e", t=SUB)
    # mask of top-8 positions (logits >= per-sub-tile 8th largest), bf16 0/1
    kth = top8_blk[:, TOP_K - 1 :: TOP_K].to_broadcast([128, SUB, N_EXPERTS])
    nc.vector.tensor_tensor(
        em3[:, :, N_EXPERTS:], logits3, kth, op=mybir.AluOpType.is_ge
    )
    # exp(l) for the aux-loss stats (bf16)
    nc.scalar.activation(
        em3[:, :, 0:N_EXPERTS], logits3, mybir.ActivationFunctionType.Exp
    )
    denom_blk = tok_pool.tile([128, SUB], F32)
    nc.vector.tensor_reduce(
        denom_blk[:],
        em3[:, :, 0:N_EXPERTS],
        axis=mybir.AxisListType.X,
        op=mybir.AluOpType.add,
    )
    recip_blk = tok_pool.tile([128, SUB], F32)
    nc.vector.reciprocal(recip_blk[:], denom_blk[:])
    ro_blk = tok_pool.tile([128, SUB * 2], BF16)
    nc.vector.memset(ro_blk[:], 1.0)
    nc.vector.tensor_copy(ro_blk[:, 0 : 2 * SUB : 2], recip_blk[:])

    # top-8 weights in f32
    e8_blk = tok_pool.tile([128, SUB * TOP_K], F32)
    nc.scalar.activation(e8_blk[:], top8_blk[:], mybir.ActivationFunctionType.Exp)
    s8_blk = tok_pool.tile([128, SUB], F32)
    nc.vector.tensor_reduce(
        s8_blk[:],
        e8_blk[:].rearrange("p (t k) -> p t k", t=SUB),
        axis=mybir.AxisListType.X,
        op=mybir.AluOpType.add,
    )
    r8_blk = tok_pool.tile([128, SUB], F32)
    nc.vector.reciprocal(r8_blk[:], s8_blk[:])
    w8_blk = out_pool.tile([128, SUB * TOP_K], F32)
    nc.vector.tensor_tensor(
        w8_blk[:].rearrange("p (t k) -> p t k", t=SUB),
        e8_blk[:].rearrange("p (t k) -> p t k", t=SUB),
        r8_blk[:].to_broadcast([128, SUB, TOP_K]),
        op=mybir.AluOpType.mult,
    )
    return dict(w8=w8_blk, idx=idx_blk, em=em_blk, ro=ro_blk)


class _LeanTileContext(tile.TileContext):
    # one end barrier instead of two: drain already waits for all sem
    # targets; the single barrier keeps sem-clears from racing pending
    # waiters, and nothing runs after the clears but engine halt.
    def _drain_and_barrier(self, tick_clock, wait_clock):
        from concourse.tile import ScopedClock

        drain_inst = self.nc.sync.drain()
        wait_clock.add_sem_waits(
            drain_inst.ins, ScopedClock({None: tick_clock.global_clock})
        )
        self.nc.all_engine_barrier()
        popped = self.nc._tile_sem_poison_stack.pop()
        assert popped is self._sem_poison
        self.nc.clear_and_free_semaphores(list(self.sems.allocated().values()))


def build_nc():
    nc = bacc.Bacc(
        "TRN2",
        target_bir_lowering=False,
        debug=False,
        enable_asserts=False,
        num_devices=N_CORES,
    )

    # packed per-(pair,half) byte planes; pair row ki holds
    # [hiA fp16 | hiB fp16 | lo pair-interleaved fp8] = 6*HALF bytes
    N_PAIRS = K_CHUNKS // 2
    xpk = nc.dram_tensor(
        "xpk", [N_PAIRS * 128, N_HALF * 6 * HALF], mybir.dt.uint8, kind="ExternalInput"
    ).ap()
    # pre-laid in SBUF order; first-4-chunk tiles separate so MMs start early
    wpka_d = nc.dram_tensor("wpka", [128, 8 * 128], F16, kind="ExternalInput").ap()
    wpkb_d = nc.dram_tensor(
        "wpkb", [128, (K_CHUNKS - 8) * 128], F16, kind="ExternalInput"
    ).ap()
    # DoubleRow weight pairs: per pair, [two=2, 64] fp8
    wl8a_d = nc.dram_tensor(
        "wl8a", [128, 4 * 2 * N_EXPERTS], F8, kind="ExternalInput"
    ).ap()
    wl8b_d = nc.dram_tensor(
        "wl8b", [128, (K_CHUNKS // 2 - 4) * 2 * N_EXPERTS], F8, kind="ExternalInput"
    ).ap()
    bias2 = nc.dram_tensor("bias2", [128, 1], F32, kind="ExternalInput").ap()
    identa = nc.dram_tensor("identa", [128, N_EXPERTS], F32, kind="ExternalInput").ap()

    w_out = nc.dram_tensor("topk_w", [T_CORE, TOP_K], F32, kind="ExternalOutput").ap()
    i_out = nc.dram_tensor("topk_i", [T_CORE, TOP_K], U32, kind="ExternalOutput").ap()
    s_out = nc.dram_tensor("stats", [128, 2], F32, kind="ExternalOutput").ap()

    with _LeanTileContext(nc) as tc:
        with (
            tc.tile_pool(name="const", bufs=1) as const_pool,
            tc.tile_pool(name="xt", bufs=12) as xt_pool,
            tc.tile_pool(name="lt", bufs=2) as lt_pool,
            tc.tile_pool(name="tok", bufs=2) as tok_pool,
            tc.tile_pool(name="outs", bufs=2) as out_pool,
            tc.tile_pool(name="psum_lt", bufs=2, space="PSUM") as psum_lt_pool,
            tc.tile_pool(name="psum_l", bufs=1, space="PSUM") as psum_l_pool,
            tc.tile_pool(name="psum_s", bufs=1, space="PSUM") as psum_s_pool,
        ):
            # --- constants ---
            # weights arrive host-pre-laid-out in SBUF order (contiguous
            # lines); first 4 chunks in their own tiles so MMs start early
            CS_A = 8
            wpk_a = const_pool.tile([128, CS_A * 128], F16)
            nc.sync.dma_start(wpk_a[:], wpka_d[:])
            wlo8_a = const_pool.tile([128, 4 * 2 * N_EXPERTS], F8)
            nc.sync.dma_start(wlo8_a[:], wl8a_d[:])
            wpk_b = const_pool.tile([128, (K_CHUNKS - CS_A) * 128], F16)
            wlo8_b = const_pool.tile(
                [128, (K_CHUNKS // 2 - 4) * 2 * N_EXPERTS], F8
            )

            def wsl_of(c):
                if c < CS_A:
                    return wpk_a[:, c * 128 : (c + 1) * 128]
                return wpk_b[:, (c - CS_A) * 128 : (c - CS_A + 1) * 128]

            def w8dr_of(p):
                # (128, 2, 64) fp8 DoubleRow weights for pair p
                w = 2 * N_EXPERTS
                if p < 4:
                    ap = wlo8_a[:, p * w : (p + 1) * w]
                else:
                    ap = wlo8_b[:, (p - 4) * w : (p - 4 + 1) * w]
                return ap.rearrange("p (two e) -> p two e", two=2)
            bias2_sb = const_pool.tile([128, 1], F32)
            identa_sb = const_pool.tile([128, N_EXPERTS], F32)
            ones_sb = const_pool.tile([128, 1], BF16)
            nc.vector.memset(ones_sb[:], 1.0)
            # stats accumulator in SBUF: col0 = sum_t score, col1 = counts
            acc_sb = const_pool.tile([128, 2], F32)
            nc.vector.memset(acc_sb[:], 0.0)

            half_state = {}

            def emit_post(h):
                ps, pslo = half_state.pop(h)
                for b in range(HALF // BLK):
                    tok0 = h * HALF + b * BLK
                    lt2_sb = lt_pool.tile(
                        [128, BLK], F32, name=f"lt2_{b}", tag=f"lt2_{b}"
                    )
                    nc.vector.tensor_scalar(
                        lt2_sb[:], ps[b][:], bias2_sb[:], None,
                        op0=mybir.AluOpType.add,
                    )
                    nc.vector.scalar_tensor_tensor(
                        out=lt2_sb[0:N_EXPERTS, :],
                        in0=pslo[b][:],
                        scalar=1.0 / (S_LO * S_W),
                        in1=lt2_sb[0:N_EXPERTS, :],
                        op0=mybir.AluOpType.mult,
                        op1=mybir.AluOpType.add,
                    )
                    # fused transpose + hi/lo-weight fold: lt2.T @ [I; I/S_W]
                    logits_blk = tok_pool.tile(
                        [128, SUB * N_EXPERTS], F32, name="logits_blk",
                        tag="logits_blk",
                    )
                    psum_l = psum_l_pool.tile(
                        [128, SUB * N_EXPERTS], F32, name="psum_l", tag="psum_l"
                    )
                    for t in range(SUB):
                        nc.tensor.matmul(
                            psum_l[:, t * N_EXPERTS : (t + 1) * N_EXPERTS],
                            lt2_sb[:, t * 128 : (t + 1) * 128],
                            identa_sb[:],
                            start=(t == 0),
                            stop=(t == SUB - 1),
                            skip_group_check=True,
                        )
                    nc.vector.tensor_copy(logits_blk[:], psum_l[:])

                    r = _topk_block(nc, (tok_pool, out_pool), logits_blk, SUB)

                    psum_s = psum_s_pool.tile([128, 2], F32, name="psum_s", tag="psum_s")
                    for t in range(SUB):
                        nc.tensor.matmul(
                            psum_s[:],
                            r["em"][:, t * 128 : (t + 1) * 128],
                            r["ro"][:, t * 2 : (t + 1) * 2],
                            start=(t == 0),
                            stop=(t == SUB - 1),
                            skip_group_check=True,
                        )
                    nc.vector.tensor_add(acc_sb[:], acc_sb[:], psum_s[:])

                    nc.sync.dma_start(
                        w_out[tok0 : tok0 + BLK, :].rearrange(
                            "(t p) k -> p t k", p=128
                        ),
                        r["w8"][:].rearrange("p (t k) -> p t k", t=SUB),
                    )
                    nc.sync.dma_start(
                        i_out[tok0 : tok0 + BLK, :].rearrange(
                            "(t p) k -> p t k", p=128
                        ),
                        r["idx"][:].rearrange("p (t k) -> p t k", t=SUB),
                    )

            for h in range(N_HALF):
                ps = [
                    psum_lt_pool.tile(
                        [128, BLK], F32, name=f"ps{b}", tag=f"ps{b}",
                        bufs=2 if b == 0 else 1,
                    )
                    for b in range(HALF // BLK)
                ]
                pslo = [
                    psum_lt_pool.tile(
                        [N_EXPERTS, BLK], F32, name=f"pslo{b}", tag=f"pslo{b}", bufs=1
                    )
                    for b in range(HALF // BLK)
                ]
                half_state[h] = (ps, pslo)
                for p in range(K_CHUNKS // 2):
                    xpk_t = xt_pool.tile([128, 6 * HALF], mybir.dt.uint8)
                    dma_eng = nc.sync if p % 2 == 0 else nc.scalar
                    dma_eng.dma_start(
                        xpk_t[:],
                        xpk[p * 128 : (p + 1) * 128, h * 6 * HALF : (h + 1) * 6 * HALF],
                    )
                    xhia_t = xpk_t[:, 0 : 2 * HALF].bitcast(F16)
                    xhib_t = xpk_t[:, 2 * HALF : 4 * HALF].bitcast(F16)
                    xlo_t = (
                        xpk_t[:, 4 * HALF : 6 * HALF]
                        .bitcast(F8)
                        .rearrange("p (two n) -> p two n", two=2)
                    )
                    if h == 0 and p == 1:
                        nc.sync.dma_start(wpk_b[:], wpkb_d[:])
                        nc.sync.dma_start(wlo8_b[:], wl8b_d[:])
                        nc.sync.dma_start(bias2_sb[:], bias2[:])
                        nc.sync.dma_start(identa_sb[:], identa[:])
                    for b in range(HALF // BLK):
                        nc.tensor.matmul(
                            ps[b][:],
                            wsl_of(2 * p),
                            xhia_t[:, b * BLK : (b + 1) * BLK],
                            start=(p == 0),
                            stop=False,
                            skip_group_check=True,
                        )
                    for b in range(HALF // BLK):
                        nc.tensor.matmul(
                            ps[b][:],
                            wsl_of(2 * p + 1),
                            xhib_t[:, b * BLK : (b + 1) * BLK],
                            start=False,
                            stop=(p == K_CHUNKS // 2 - 1),
                            skip_group_check=True,
                        )
                    for b in range(HALF // BLK):
                        nc.tensor.matmul(
                            pslo[b][:],
                            w8dr_of(p),
                            xlo_t[:, :, b * BLK : (b + 1) * BLK],
                            start=(p == 0),
                            stop=(p == K_CHUNKS // 2 - 1),
                            skip_group_check=True,
                            perf_mode=mybir.MatmulPerfMode.DoubleRow,
                        )
                emit_post(h)

            nc.sync.dma_start(s_out[:], acc_sb[:])

    nc.compile()
    return nc


def _get_nc():
    global _CACHED_NC
    if _CACHED_NC is None:
        _CACHED_NC = build_nc()
    return _CACHED_NC


def kernel(hidden_states, router_weight, adaptive_bias, expert_quality_ema):
    global LAST_RESULTS
    import ml_dtypes

    f8 = ml_dtypes.float8_e4m3
    X = np.asarray(hidden_states, dtype=np.float32)
    W = np.asarray(router_weight, dtype=np.float32)
    ab = np.asarray(adaptive_bias, dtype=np.float32)
    ema = np.asarray(expert_quality_ema, dtype=np.float32)

    qb = ema / max(float(np.linalg.norm(ema)), 1e-12)
    bias2 = np.zeros((128, 1), dtype=np.float32)
    bias2[:N_EXPERTS, 0] = ab + qb
    wT = np.ascontiguousarray(W.T)  # (HIDDEN, 64) f32
    w_hi = wT.astype(np.float16)
    w_lo = ((wT - w_hi.astype(np.float32)) * S_W).astype(np.float16)
    wpk0 = np.concatenate([w_hi, w_lo], axis=1)  # (HIDDEN, 128) fp16
    wlo80 = (wT * S_W).astype(f8)  # (HIDDEN, 64) fp8

    def _sbuf_layout(warr, width, split):
        t = warr.reshape(-1, 128, width).transpose(1, 0, 2)
        a = np.ascontiguousarray(t[:, :split, :]).reshape(128, split * width)
        b = np.ascontiguousarray(t[:, split:, :]).reshape(128, -1)
        return a, b

    wpka, wpkb = _sbuf_layout(wpk0, 128, 8)
    # DoubleRow pair layout: (128, pair, two, 64)
    w8t = wlo80.reshape(K_CHUNKS // 2, 2, 128, N_EXPERTS).transpose(2, 0, 1, 3)
    wl8a = np.ascontiguousarray(w8t[:, :4]).reshape(128, -1)
    wl8b = np.ascontiguousarray(w8t[:, 4:]).reshape(128, -1)
    eye = np.eye(N_EXPERTS, dtype=np.float32)
    identa = np.vstack([eye, eye / S_W])
    xT = X.T  # (HIDDEN, N_TOKENS) view

    in_maps = []
    for c in range(N_CORES):
        shard = np.ascontiguousarray(xT[:, c * T_CORE : (c + 1) * T_CORE])
        s_hi = shard.astype(np.float16)
        s_lo = ((shard - s_hi.astype(np.float32)) * S_LO).astype(f8)
        # pair-major rows: row p*128+ki holds hiA/hiB fp16 + pair-interleaved fp8
        hi4 = s_hi.reshape(K_CHUNKS // 2, 2, 128, T_CORE)
        lo4 = s_lo.reshape(K_CHUNKS // 2, 2, 128, T_CORE)
        xpk = np.empty((K_CHUNKS // 2 * 128, N_HALF * 6 * HALF), dtype=np.uint8)
        xpk4 = xpk.reshape(K_CHUNKS // 2, 128, N_HALF, 6 * HALF)
        for h in range(N_HALF):
            tsl = slice(h * HALF, (h + 1) * HALF)
            xpk4[:, :, h, 0 : 2 * HALF] = (
                np.ascontiguousarray(hi4[:, 0, :, tsl]).view(np.uint8)
            )
            xpk4[:, :, h, 2 * HALF : 4 * HALF] = (
                np.ascontiguousarray(hi4[:, 1, :, tsl]).view(np.uint8)
            )
            xpk4[:, :, h, 4 * HALF : 5 * HALF] = (
                np.ascontiguousarray(lo4[:, 0, :, tsl]).view(np.uint8)
            )
            xpk4[:, :, h, 5 * HALF : 6 * HALF] = (
                np.ascontiguousarray(lo4[:, 1, :, tsl]).view(np.uint8)
            )
        in_maps.append(
            {
                "xpk": xpk,
                "wpka": wpka,
                "wpkb": wpkb,
                "wl8a": wl8a,
                "wl8b": wl8b,
                "bias2": bias2,
                "identa": identa,
            }
        )

    nc = _get_nc()
    trace = bool(os.environ.get("BASS_TRACE")) and _install_ntff_hook()
    try:
        res = run_bass_kernel_spmd(
            nc, in_maps, core_ids=list(range(N_CORES)), trace=trace
        )
    except Exception:
        if not trace:
            raise
        os.environ["BASS_NEVER_TRACE"] = "1"
        res = run_bass_kernel_spmd(
            nc, in_maps, core_ids=list(range(N_CORES)), trace=False
        )
    LAST_RESULTS = res

    topk_w = np.concatenate([r["topk_w"] for r in res.results], axis=0)
    topk_i = np.concatenate([r["topk_i"] for r in res.results], axis=0).astype(np.int32)
    stats = np.stack([r["stats"] for r in res.results]).sum(axis=0)  # (128, 2)
    mean_prob = stats[:N_EXPERTS, 0] / float(N_TOKENS)
    tokens_per_expert = stats[N_EXPERTS:, 1] / float(N_TOKENS * TOP_K)
    aux = np.float32(N_EXPERTS * np.sum(tokens_per_expert * mean_prob))
    return topk_w, topk_i, aux


# revision 38
# speedup vs baseline: 1.0457x; 1.0457x over previous
"""Trainium2 Bass kernel for AdaptiveRouter MoE routing.

reference:
  logits = hidden @ W^T + adaptive_bias + ema/||ema||  (N=16384, H=4096, E=64)
  scores = softmax(logits); topk_w, topk_i = top_k(scores, 8); topk_w /= sum
  aux = E * sum(tokens_per_expert * mean_prob)

Sharding: data-parallel over tokens across 8 cores (2048 tokens each);
router weight/bias replicated. Aux-loss partials (per-expert score sums,
per-expert top-8 counts) are computed per core and combined on the host
during the gather/unshard step.

Numerics / layout strategy (device is memory-bound; fp32 PE matmul runs
in slow 2-pass LOW_HIGH mode, and X^T is needed because the PE contracts
along partitions):
  - host transposes X and encodes each element in 3 bytes: fp16 hi plane
    plus fp8e4m3 residual plane scaled by 1024 (max logit err ~3.5e-5 ->
    0 of 131072 top-8 indices flip vs the fp32 reference on this data)
  - weights: [fp16(W^T) | (W^T - fp16(W^T))*64] packed 128-col stationary
    for the hi pass; fp8(W^T*64) in DoubleRow pair layout for the lo pass
  - the two planes are packed per (chunk-pair, token-half) into one uint8
    DRAM buffer so each (128, 6KB contiguous-line) tile arrives in ONE
    DMA (descriptor generation, ~0.7us per 128-line DMA, would otherwise
    pace the stream); on-chip views via AP.bitcast
  - hi pass: 2 fp16 MMs per pair into a (128, 512) PSUM block (rows 0-63
    accumulate W_hi terms, 64-127 W_lo terms); lo pass: 1 fp8 DoubleRow
    MM per pair (contracts both chunks at 2 elements/lane/cycle) into a
    (64, 512) PSUM
  - evacuation adds [bias; 0] (per-partition scalar) and folds the lo
    PSUM into rows 0-63 (scalar_tensor_tensor); a fp32 matmul against
    [I; I/64] then transposes to (token, expert) while summing the hi/lo
    weight rows
  - nc.vector.max / max_index give the top-8 (descending) per token;
    softmax needs no max subtraction (logits are O(6), fp32-safe);
    topk_w = exp(top8) / sum(exp(top8))
  - stats: one fused matmul per 128-token tile with lhsT = [exp(l) |
    top8-mask] bf16 and rhs = [1/denom | 1], contracting over the token
    partition dim; accumulated in PSUM/SBUF and summed on host
"""

import os

import numpy as np

import concourse.bass as bass
import concourse.mybir as mybir
from concourse import bacc
from concourse import tile
from concourse.bass_utils import run_bass_kernel_spmd

F32 = mybir.dt.float32
BF16 = mybir.dt.bfloat16
F16 = mybir.dt.float16
F8 = mybir.dt.float8e4
U32 = mybir.dt.uint32
S_LO = 1024.0  # host scale on the fp8 residual plane
S_W = 64.0  # host scale on the W-residual / fp8-W planes

N_TOKENS = 16384
HIDDEN = 4096
N_EXPERTS = 64
TOP_K = 8
N_CORES = 8
T_CORE = N_TOKENS // N_CORES  # 2048
K_CHUNKS = HIDDEN // 128  # 32
BLK = 512  # tokens per PSUM block
HALF = 1024  # tokens per outer iteration (2 PSUM blocks)
N_HALF = T_CORE // HALF  # 2
SUB = BLK // 128  # 4 sub-tiles of 128 tokens per block

_CACHED_NC = None
LAST_RESULTS = None


def _install_ntff_hook():
    """concourse's traced axon path imports antenv.axon_hooks, which this
    image lacks. Recreate it in sys.modules and register the ctypes-based
    NTFF profile hook from trn_agent_boot."""
    import sys
    import types

    if "antenv.axon_hooks" in sys.modules:
        return True
    try:
        import antenv
        from trn_agent_boot.trn_boot import _ntff_profile_via_ctypes

        mod = types.ModuleType("antenv.axon_hooks")
        mod._hook = _ntff_profile_via_ctypes("/opt/axon/libaxon_pjrt.so")

        def set_axon_ntff_profile_hook(h):
            mod._hook = h

        def get_axon_ntff_profile_hook():
            return mod._hook

        mod.set_axon_ntff_profile_hook = set_axon_ntff_profile_hook
        mod.get_axon_ntff_profile_hook = get_axon_ntff_profile_hook
        sys.modules["antenv.axon_hooks"] = mod
        antenv.axon_hooks = mod
        return True
    except Exception:
        return False


def _topk_block(nc, pools, logits_blk, nsub):
    """Top-8 + softmax + stats inputs over nsub 128-token sub-tiles.

    logits_blk: (128, nsub*64) f32 SBUF tile of logits in (token, expert)
    layout. Produces output tiles plus fused stats operands:
    em = [exp(l) | top8-mask] (128, nsub*128) bf16, ro = [recip | 1] pairs.
    """
    SUB = nsub
    tok_pool, out_pool = pools
    top8_blk = tok_pool.tile([128, SUB * TOP_K], F32)
    idx_blk = out_pool.tile([128, SUB * TOP_K], U32)
    em_blk = tok_pool.tile([128, SUB * 128], BF16)
    em3 = em_blk[:].rearrange("p (t c) -> p t c", t=SUB)
    for t in range(SUB):
        lsl = logits_blk[:, t * N_EXPERTS : (t + 1) * N_EXPERTS]
        t8 = top8_blk[:, t * TOP_K : (t + 1) * TOP_K]
        nc.vector.max(out=t8, in_=lsl)
        nc.vector.max_index(
            out=idx_blk[:, t * TOP_K : (t + 1) * TOP_K], in_max=t8, in_values=lsl
        )
    logits3 = logits_blk[:].rearrange("p (t e) -> p t e", t=SUB)
    # mask of top-8 positions (logits >= per-sub-tile 8th largest), bf16 0/1
    kth = top8_blk[:, TOP_K - 1 :: TOP_K].to_broadcast([128, SUB, N_EXPERTS])
    nc.vector.tensor_tensor(
        em3[:, :, N_EXPERTS:], logits3, kth, op=mybir.AluOpType.is_ge
    )
    # exp(l) for the aux-loss stats (bf16)
    nc.scalar.activation(
        em3[:, :, 0:N_EXPERTS], logits3, mybir.ActivationFunctionType.Exp
    )
    denom_blk = tok_pool.tile([128, SUB], F32)
    nc.vector.tensor_reduce(
        denom_blk[:],
        em3[:, :, 0:N_EXPERTS],
        axis=mybir.AxisListType.X,
        op=mybir.AluOpType.add,
    )
    recip_blk = tok_pool.tile([128, SUB], F32)
    nc.vector.reciprocal(recip_blk[:], denom_blk[:])
    ro_blk = tok_pool.tile([128, SUB * 2], BF16)
    nc.vector.memset(ro_blk[:], 1.0)
    nc.vector.tensor_copy(ro_blk[:, 0 : 2 * SUB : 2], recip_blk[:])

    # top-8 weights in f32
    e8_blk = tok_pool.tile([128, SUB * TOP_K], F32)
    nc.scalar.activation(e8_blk[:], top8_blk[:], mybir.ActivationFunctionType.Exp)
    s8_blk = tok_pool.tile([128, SUB], F32)
    nc.vector.tensor_reduce(
        s8_blk[:],
        e8_blk[:].rearrange("p (t k) -> p t k", t=SUB),
        axis=mybir.AxisListType.X,
        op=mybir.AluOpType.add,
    )
    r8_blk = tok_pool.tile([128, SUB], F32)
    nc.vector.reciprocal(r8_blk[:], s8_blk[:])
    w8_blk = out_pool.tile([128, SUB * TOP_K], F32)
    nc.vector.tensor_tensor(
        w8_blk[:].rearrange("p (t k) -> p t k", t=SUB),
        e8_blk[:].rearrange("p (t k) -> p t k", t=SUB),
        r8_blk[:].to_broadcast([128, SUB, TOP_K]),
        op=mybir.AluOpType.mult,
    )
    return dict(w8=w8_blk, idx=idx_blk, em=em_blk, ro=ro_blk)


class _LeanTileContext(tile.TileContext):
    # one end barrier instead of two: drain already waits for all sem
    # targets; the single barrier keeps sem-clears from racing pending
    # waiters, and nothing runs after the clears but engine halt.
    def _drain_and_barrier(self, tick_clock, wait_clock):
        from concourse.tile import ScopedClock

        drain_inst = self.nc.sync.drain()
        wait_clock.add_sem_waits(
            drain_inst.ins, ScopedClock({None: tick_clock.global_clock})
        )
        self.nc.all_engine_barrier()
        popped = self.nc._tile_sem_poison_stack.pop()
        assert popped is self._sem_poison
        self.nc.clear_and_free_semaphores(list(self.sems.allocated().values()))


def build_nc():
    nc = bacc.Bacc(
        "TRN2",
        target_bir_lowering=False,
        debug=False,
        enable_asserts=False,
        num_devices=N_CORES,
    )

    # packed per-(pair,half) byte planes; pair row ki holds
    # [hiA fp16 | hiB fp16 | lo pair-interleaved fp8] = 6*HALF bytes
    N_PAIRS = K_CHUNKS // 2
    xpk = nc.dram_tensor(
        "xpk", [N_PAIRS * 128, N_HALF * 6 * HALF], mybir.dt.uint8, kind="ExternalInput"
    ).ap()
    # pre-laid in SBUF order; first-4-chunk tiles separate so MMs start early
    wpka_d = nc.dram_tensor("wpka", [128, 8 * 128], F16, kind="ExternalInput").ap()
    wpkb_d = nc.dram_tensor(
        "wpkb", [128, (K_CHUNKS - 8) * 128], F16, kind="ExternalInput"
    ).ap()
    # DoubleRow weight pairs: per pair, [two=2, 64] fp8
    wl8a_d = nc.dram_tensor(
        "wl8a", [128, 4 * 2 * N_EXPERTS], F8, kind="ExternalInput"
    ).ap()
    wl8b_d = nc.dram_tensor(
        "wl8b", [128, (K_CHUNKS // 2 - 4) * 2 * N_EXPERTS], F8, kind="ExternalInput"
    ).ap()
    bias2 = nc.dram_tensor("bias2", [128, 1], F32, kind="ExternalInput").ap()
    identa = nc.dram_tensor("identa", [128, N_EXPERTS], F32, kind="ExternalInput").ap()

    w_out = nc.dram_tensor("topk_w", [T_CORE, TOP_K], F32, kind="ExternalOutput").ap()
    i_out = nc.dram_tensor("topk_i", [T_CORE, TOP_K], U32, kind="ExternalOutput").ap()
    s_out = nc.dram_tensor("stats", [128, 2], F32, kind="ExternalOutput").ap()

    with _LeanTileContext(nc) as tc:
        with (
            tc.tile_pool(name="const", bufs=1) as const_pool,
            tc.tile_pool(name="xt", bufs=12) as xt_pool,
            tc.tile_pool(name="lt", bufs=2) as lt_pool,
            tc.tile_pool(name="tok", bufs=2) as tok_pool,
            tc.tile_pool(name="outs", bufs=2) as out_pool,
            tc.tile_pool(name="psum_lt", bufs=2, space="PSUM") as psum_lt_pool,
            tc.tile_pool(name="psum_l", bufs=1, space="PSUM") as psum_l_pool,
            tc.tile_pool(name="psum_s", bufs=1, space="PSUM") as psum_s_pool,
        ):
            # --- constants ---
            # weights arrive host-pre-laid-out in SBUF order (contiguous
            # lines); first 4 chunks in their own tiles so MMs start early
            CS_A = 8
            wpk_a = const_pool.tile([128, CS_A * 128], F16)
            nc.sync.dma_start(wpk_a[:], wpka_d[:])
            wlo8_a = const_pool.tile([128, 4 * 2 * N_EXPERTS], F8)
            nc.sync.dma_start(wlo8_a[:], wl8a_d[:])
            wpk_b = const_pool.tile([128, (K_CHUNKS - CS_A) * 128], F16)
            wlo8_b = const_pool.tile(
                [128, (K_CHUNKS // 2 - 4) * 2 * N_EXPERTS], F8
            )

            def wsl_of(c):
                if c < CS_A:
                    return wpk_a[:, c * 128 : (c + 1) * 128]
                return wpk_b[:, (c - CS_A) * 128 : (c - CS_A + 1) * 128]

            def w8dr_of(p):
                # (128, 2, 64) fp8 DoubleRow weights for pair p
                w = 2 * N_EXPERTS
                if p < 4:
                    ap = wlo8_a[:, p * w : (p + 1) * w]
                else:
                    ap = wlo8_b[:, (p - 4) * w : (p - 4 + 1) * w]
                return ap.rearrange("p (two e) -> p two e", two=2)
            bias2_sb = const_pool.tile([128, 1], F32)
            identa_sb = const_pool.tile([128, N_EXPERTS], F32)
            ones_sb = const_pool.tile([128, 1], BF16)
            nc.vector.memset(ones_sb[:], 1.0)
            # stats accumulator in SBUF: col0 = sum_t score, col1 = counts
            acc_sb = const_pool.tile([128, 2], F32)
            nc.vector.memset(acc_sb[:], 0.0)

            half_state = {}

            def emit_post(h):
                ps, pslo = half_state.pop(h)
                for b in range(HALF // BLK):
                    tok0 = h * HALF + b * BLK
                    lt2_sb = lt_pool.tile(
                        [128, BLK], F32, name=f"lt2_{b}", tag=f"lt2_{b}"
                    )
                    nc.vector.tensor_scalar(
                        lt2_sb[:], ps[b][:], bias2_sb[:], None,
                        op0=mybir.AluOpType.add,
                    )
                    nc.vector.scalar_tensor_tensor(
                        out=lt2_sb[0:N_EXPERTS, :],
                        in0=pslo[b][:],
                        scalar=1.0 / (S_LO * S_W),
                        in1=lt2_sb[0:N_EXPERTS, :],
                        op0=mybir.AluOpType.mult,
                        op1=mybir.AluOpType.add,
                    )
                    # fused transpose + hi/lo-weight fold: lt2.T @ [I; I/S_W]
                    logits_blk = tok_pool.tile(
                        [128, SUB * N_EXPERTS], F32, name="logits_blk",
                        tag="logits_blk",
                    )
                    psum_l = psum_l_pool.tile(
                        [128, SUB * N_EXPERTS], F32, name="psum_l", tag="psum_l"
                    )
                    for t in range(SUB):
                        nc.tensor.matmul(
                            psum_l[:, t * N_EXPERTS : (t + 1) * N_EXPERTS],
                            lt2_sb[:, t * 128 : (t + 1) * 128],
                            identa_sb[:],
                            start=(t == 0),
                            stop=(t == SUB - 1),
                            skip_group_check=True,
                        )
                    nc.vector.tensor_copy(logits_blk[:], psum_l[:])

                    r = _topk_block(nc, (tok_pool, out_pool), logits_blk, SUB)

                    psum_s = psum_s_pool.tile([128, 2], F32, name="psum_s", tag="psum_s")
                    for t in range(SUB):
                        nc.tensor.matmul(
                            psum_s[:],
                            r["em"][:, t * 128 : (t + 1) * 128],
                            r["ro"][:, t * 2 : (t + 1) * 2],
                            start=(t == 0),
                            stop=(t == SUB - 1),
                            skip_group_check=True,
                        )
                    nc.vector.tensor_add(acc_sb[:], acc_sb[:], psum_s[:])

                    nc.sync.dma_start(
                        w_out[tok0 : tok0 + BLK, :].rearrange(
                            "(t p) k -> p t k", p=128
                        ),
                        r["w8"][:].rearrange("p (t k) -> p t k", t=SUB),
                    )
                    nc.sync.dma_start(
                        i_out[tok0 : tok0 + BLK, :].rearrange(
                            "(t p) k -> p t k", p=128
                        ),
                        r["idx"][:].rearrange("p (t k) -> p t k", t=SUB),
                    )

            for h in range(N_HALF):
                ps = [
                    psum_lt_pool.tile(
                        [128, BLK], F32, name=f"ps{b}", tag=f"ps{b}",
                        bufs=2 if b == 0 else 1,
                    )
                    for b in range(HALF // BLK)
                ]
                pslo = [
                    psum_lt_pool.tile(
                        [N_EXPERTS, BLK], F32, name=f"pslo{b}", tag=f"pslo{b}", bufs=1
                    )
                    for b in range(HALF // BLK)
                ]
                half_state[h] = (ps, pslo)
                for p in range(K_CHUNKS // 2):
                    xpk_t = xt_pool.tile([128, 6 * HALF], mybir.dt.uint8)
                    nc.sync.dma_start(
                        xpk_t[:],
                        xpk[p * 128 : (p + 1) * 128, h * 6 * HALF : (h + 1) * 6 * HALF],
                    )
                    xhia_t = xpk_t[:, 0 : 2 * HALF].bitcast(F16)
                    xhib_t = xpk_t[:, 2 * HALF : 4 * HALF].bitcast(F16)
                    xlo_t = (
                        xpk_t[:, 4 * HALF : 6 * HALF]
                        .bitcast(F8)
                        .rearrange("p (two n) -> p two n", two=2)
                    )
                    if h == 0 and p == 1:
                        nc.sync.dma_start(wpk_b[:], wpkb_d[:])
                        nc.sync.dma_start(wlo8_b[:], wl8b_d[:])
                        nc.sync.dma_start(bias2_sb[:], bias2[:])
                        nc.sync.dma_start(identa_sb[:], identa[:])
                    for b in range(HALF // BLK):
                        nc.tensor.matmul(
                            ps[b][:],
                            wsl_of(2 * p),
                            xhia_t[:, b * BLK : (b + 1) * BLK],
                            start=(p == 0),
                            stop=False,
                            skip_group_check=True,
                        )
                    for b in range(HALF // BLK):
                        nc.tensor.matmul(
                            ps[b][:],
                            wsl_of(2 * p + 1),
                            xhib_t[:, b * BLK : (b + 1) * BLK],
                            start=False,
                            stop=(p == K_CHUNKS // 2 - 1),
                            skip_group_check=True,
                        )
                    for b in range(HALF // BLK):
                        nc.tensor.matmul(
                            pslo[b][:],
                            w8dr_of(p),
                            xlo_t[:, :, b * BLK : (b + 1) * BLK],
                            start=(p == 0),
                            stop=(p == K_CHUNKS // 2 - 1),
                            skip_group_check=True,
                            perf_mode=mybir.MatmulPerfMode.DoubleRow,
                        )
                emit_post(h)

            nc.sync.dma_start(s_out[:], acc_sb[:])

    nc.compile()
    return nc


def _get_nc():
    global _CACHED_NC
    if _CACHED_NC is None:
        _CACHED_NC = build_nc()
    return _CACHED_NC


def kernel(hidden_states, router_weight, adaptive_bias, expert_quality_ema):
    global LAST_RESULTS
    import ml_dtypes

    f8 = ml_dtypes.float8_e4m3
    X = np.asarray(hidden_states, dtype=np.float32)
    W = np.asarray(router_weight, dtype=np.float32)
    ab = np.asarray(adaptive_bias, dtype=np.float32)
    ema = np.asarray(expert_quality_ema, dtype=np.float32)

    qb = ema / max(float(np.linalg.norm(ema)), 1e-12)
    bias2 = np.zeros((128, 1), dtype=np.float32)
    bias2[:N_EXPERTS, 0] = ab + qb
    wT = np.ascontiguousarray(W.T)  # (HIDDEN, 64) f32
    w_hi = wT.astype(np.float16)
    w_lo = ((wT - w_hi.astype(np.float32)) * S_W).astype(np.float16)
    wpk0 = np.concatenate([w_hi, w_lo], axis=1)  # (HIDDEN, 128) fp16
    wlo80 = (wT * S_W).astype(f8)  # (HIDDEN, 64) fp8

    def _sbuf_layout(warr, width, split):
        t = warr.reshape(-1, 128, width).transpose(1, 0, 2)
        a = np.ascontiguousarray(t[:, :split, :]).reshape(128, split * width)
        b = np.ascontiguousarray(t[:, split:, :]).reshape(128, -1)
        return a, b

    wpka, wpkb = _sbuf_layout(wpk0, 128, 8)
    # DoubleRow pair layout: (128, pair, two, 64)
    w8t = wlo80.reshape(K_CHUNKS // 2, 2, 128, N_EXPERTS).transpose(2, 0, 1, 3)
    wl8a = np.ascontiguousarray(w8t[:, :4]).reshape(128, -1)
    wl8b = np.ascontiguousarray(w8t[:, 4:]).reshape(128, -1)
    eye = np.eye(N_EXPERTS, dtype=np.float32)
    identa = np.vstack([eye, eye / S_W])
    xT = X.T  # (HIDDEN, N_TOKENS) view

    in_maps = []
    for c in range(N_CORES):
        shard = np.ascontiguousarray(xT[:, c * T_CORE : (c + 1) * T_CORE])
        s_hi = shard.astype(np.float16)
        s_lo = ((shard - s_hi.astype(np.float32)) * S_LO).astype(f8)
        # pair-major rows: row p*128+ki holds hiA/hiB fp16 + pair-interleaved fp8
        hi4 = s_hi.reshape(K_CHUNKS // 2, 2, 128, T_CORE)
        lo4 = s_lo.reshape(K_CHUNKS // 2, 2, 128, T_CORE)
        xpk = np.empty((K_CHUNKS // 2 * 128, N_HALF * 6 * HALF), dtype=np.uint8)
        xpk4 = xpk.reshape(K_CHUNKS // 2, 128, N_HALF, 6 * HALF)
        for h in range(N_HALF):
            tsl = slice(h * HALF, (h + 1) * HALF)
            xpk4[:, :, h, 0 : 2 * HALF] = (
                np.ascontiguousarray(hi4[:, 0, :, tsl]).view(np.uint8)
            )
            xpk4[:, :, h, 2 * HALF : 4 * HALF] = (
                np.ascontiguousarray(hi4[:, 1, :, tsl]).view(np.uint8)
            )
            xpk4[:, :, h, 4 * HALF : 5 * HALF] = (
                np.ascontiguousarray(lo4[:, 0, :, tsl]).view(np.uint8)
            )
            xpk4[:, :, h, 5 * HALF : 6 * HALF] = (
                np.ascontiguousarray(lo4[:, 1, :, tsl]).view(np.uint8)
            )
        in_maps.append(
            {
                "xpk": xpk,
                "wpka": wpka,
                "wpkb": wpkb,
                "wl8a": wl8a,
                "wl8b": wl8b,
                "bias2": bias2,
                "identa": identa,
            }
        )

    nc = _get_nc()
    trace = bool(os.environ.get("BASS_TRACE")) and _install_ntff_hook()
    try:
        res = run_bass_kernel_spmd(
            nc, in_maps, core_ids=list(range(N_CORES)), trace=trace
        )
    except Exception:
        if not trace:
            raise
        os.environ["BASS_NEVER_TRACE"] = "1"
        res = run_bass_kernel_spmd(
            nc, in_maps, core_ids=list(range(N_CORES)), trace=False
        )
    LAST_RESULTS = res

    topk_w = np.concatenate([r["topk_w"] for r in res.results], axis=0)
    topk_i = np.concatenate([r["topk_i"] for r in res.results], axis=0).astype(np.int32)
    stats = np.stack([r["stats"] for r in res.results]).sum(axis=0)  # (128, 2)
    mean_prob = stats[:N_EXPERTS, 0] / float(N_TOKENS)
    tokens_per_expert = stats[N_EXPERTS:, 1] / float(N_TOKENS * TOP_K)
    aux = np.float32(N_EXPERTS * np.sum(tokens_per_expert * mean_prob))
    return topk_w, topk_i, aux


# revision 39
# speedup vs baseline: 1.0765x; 1.0295x over previous
"""Trainium2 Bass kernel for AdaptiveRouter MoE routing.

reference:
  logits = hidden @ W^T + adaptive_bias + ema/||ema||  (N=16384, H=4096, E=64)
  scores = softmax(logits); topk_w, topk_i = top_k(scores, 8); topk_w /= sum
  aux = E * sum(tokens_per_expert * mean_prob)

Sharding: data-parallel over tokens across 8 cores (2048 tokens each);
router weight/bias replicated. Aux-loss partials (per-expert score sums,
per-expert top-8 counts) are computed per core and combined on the host
during the gather/unshard step.

Numerics / layout strategy (device is memory-bound; fp32 PE matmul runs
in slow 2-pass LOW_HIGH mode, and X^T is needed because the PE contracts
along partitions):
  - host transposes X and encodes each element in 3 bytes: fp16 hi plane
    plus fp8e4m3 residual plane scaled by 1024 (max logit err ~3.5e-5 ->
    0 of 131072 top-8 indices flip vs the fp32 reference on this data)
  - weights: [fp16(W^T) | (W^T - fp16(W^T))*64] packed 128-col stationary
    for the hi pass; fp8(W^T*64) in DoubleRow pair layout for the lo pass
  - the two planes are packed per (chunk-pair, token-half) into one uint8
    DRAM buffer so each (128, 6KB contiguous-line) tile arrives in ONE
    DMA (descriptor generation, ~0.7us per 128-line DMA, would otherwise
    pace the stream); on-chip views via AP.bitcast
  - hi pass: 2 fp16 MMs per pair into a (128, 512) PSUM block (rows 0-63
    accumulate W_hi terms, 64-127 W_lo terms); lo pass: 1 fp8 DoubleRow
    MM per pair (contracts both chunks at 2 elements/lane/cycle) into a
    (64, 512) PSUM
  - evacuation adds [bias; 0] (per-partition scalar) and folds the lo
    PSUM into rows 0-63 (scalar_tensor_tensor); a fp32 matmul against
    [I; I/64] then transposes to (token, expert) while summing the hi/lo
    weight rows
  - nc.vector.max / max_index give the top-8 (descending) per token;
    softmax needs no max subtraction (logits are O(6), fp32-safe);
    topk_w = exp(top8) / sum(exp(top8))
  - stats: one fused matmul per 128-token tile with lhsT = [exp(l) |
    top8-mask] bf16 and rhs = [1/denom | 1], contracting over the token
    partition dim; accumulated in PSUM/SBUF and summed on host
"""

import os

import numpy as np

import concourse.bass as bass
import concourse.mybir as mybir
from concourse import bacc
from concourse import tile
from concourse.bass_utils import run_bass_kernel_spmd

F32 = mybir.dt.float32
BF16 = mybir.dt.bfloat16
F16 = mybir.dt.float16
F8 = mybir.dt.float8e4
U32 = mybir.dt.uint32
S_LO = 1024.0  # host scale on the fp8 residual plane
S_W = 64.0  # host scale on the W-residual / fp8-W planes

N_TOKENS = 16384
HIDDEN = 4096
N_EXPERTS = 64
TOP_K = 8
N_CORES = 8
T_CORE = N_TOKENS // N_CORES  # 2048
K_CHUNKS = HIDDEN // 128  # 32
BLK = 512  # tokens per PSUM block
HALF = 1024  # tokens per outer iteration (2 PSUM blocks)
N_HALF = T_CORE // HALF  # 2
SUB = BLK // 128  # 4 sub-tiles of 128 tokens per block

_CACHED_NC = None
LAST_RESULTS = None


def _install_ntff_hook():
    """concourse's traced axon path imports antenv.axon_hooks, which this
    image lacks. Recreate it in sys.modules and register the ctypes-based
    NTFF profile hook from trn_agent_boot."""
    import sys
    import types

    if "antenv.axon_hooks" in sys.modules:
        return True
    try:
        import antenv
        from trn_agent_boot.trn_boot import _ntff_profile_via_ctypes

        mod = types.ModuleType("antenv.axon_hooks")
        mod._hook = _ntff_profile_via_ctypes("/opt/axon/libaxon_pjrt.so")

        def set_axon_ntff_profile_hook(h):
            mod._hook = h

        def get_axon_ntff_profile_hook():
            return mod._hook

        mod.set_axon_ntff_profile_hook = set_axon_ntff_profile_hook
        mod.get_axon_ntff_profile_hook = get_axon_ntff_profile_hook
        sys.modules["antenv.axon_hooks"] = mod
        antenv.axon_hooks = mod
        return True
    except Exception:
        return False


def _topk_block(nc, pools, logits_blk, nsub):
    """Top-8 + softmax + stats inputs over nsub 128-token sub-tiles.

    logits_blk: (128, nsub*64) f32 SBUF tile of logits in (token, expert)
    layout. Produces output tiles plus fused stats operands:
    em = [exp(l) | top8-mask] (128, nsub*128) bf16, ro = [recip | 1] pairs.
    """
    SUB = nsub
    tok_pool, out_pool = pools
    top8_blk = tok_pool.tile([128, SUB * TOP_K], F32)
    idx_blk = out_pool.tile([128, SUB * TOP_K], U32)
    em_blk = tok_pool.tile([128, SUB * 128], BF16)
    em3 = em_blk[:].rearrange("p (t c) -> p t c", t=SUB)
    for t in range(SUB):
        lsl = logits_blk[:, t * N_EXPERTS : (t + 1) * N_EXPERTS]
        t8 = top8_blk[:, t * TOP_K : (t + 1) * TOP_K]
        nc.vector.max(out=t8, in_=lsl)
        nc.vector.max_index(
            out=idx_blk[:, t * TOP_K : (t + 1) * TOP_K], in_max=t8, in_values=lsl
        )
    logits3 = logits_blk[:].rearrange("p (t e) -> p t e", t=SUB)
    # mask of top-8 positions (logits >= per-sub-tile 8th largest), bf16 0/1
    kth = top8_blk[:, TOP_K - 1 :: TOP_K].to_broadcast([128, SUB, N_EXPERTS])
    nc.vector.tensor_tensor(
        em3[:, :, N_EXPERTS:], logits3, kth, op=mybir.AluOpType.is_ge
    )
    # exp(l) for the aux-loss stats (bf16)
    nc.scalar.activation(
        em3[:, :, 0:N_EXPERTS], logits3, mybir.ActivationFunctionType.Exp
    )
    denom_blk = tok_pool.tile([128, SUB], F32)
    nc.vector.tensor_reduce(
        denom_blk[:],
        em3[:, :, 0:N_EXPERTS],
        axis=mybir.AxisListType.X,
        op=mybir.AluOpType.add,
    )
    recip_blk = tok_pool.tile([128, SUB], F32)
    nc.vector.reciprocal(recip_blk[:], denom_blk[:])
    ro_blk = tok_pool.tile([128, SUB * 2], BF16)
    nc.vector.memset(ro_blk[:], 1.0)
    nc.vector.tensor_copy(ro_blk[:, 0 : 2 * SUB : 2], recip_blk[:])

    # top-8 weights in f32
    e8_blk = tok_pool.tile([128, SUB * TOP_K], F32)
    nc.scalar.activation(e8_blk[:], top8_blk[:], mybir.ActivationFunctionType.Exp)
    s8_blk = tok_pool.tile([128, SUB], F32)
    nc.vector.tensor_reduce(
        s8_blk[:],
        e8_blk[:].rearrange("p (t k) -> p t k", t=SUB),
        axis=mybir.AxisListType.X,
        op=mybir.AluOpType.add,
    )
    r8_blk = tok_pool.tile([128, SUB], F32)
    nc.vector.reciprocal(r8_blk[:], s8_blk[:])
    w8_blk = out_pool.tile([128, SUB * TOP_K], F32)
    nc.vector.tensor_tensor(
        w8_blk[:].rearrange("p (t k) -> p t k", t=SUB),
        e8_blk[:].rearrange("p (t k) -> p t k", t=SUB),
        r8_blk[:].to_broadcast([128, SUB, TOP_K]),
        op=mybir.AluOpType.mult,
    )
    return dict(w8=w8_blk, idx=idx_blk, em=em_blk, ro=ro_blk)


class _LeanTileContext(tile.TileContext):
    # one end barrier instead of two: drain already waits for all sem
    # targets; the single barrier keeps sem-clears from racing pending
    # waiters, and nothing runs after the clears but engine halt.
    def _drain_and_barrier(self, tick_clock, wait_clock):
        from concourse.tile import ScopedClock

        drain_inst = self.nc.sync.drain()
        wait_clock.add_sem_waits(
            drain_inst.ins, ScopedClock({None: tick_clock.global_clock})
        )
        self.nc.all_engine_barrier()
        popped = self.nc._tile_sem_poison_stack.pop()
        assert popped is self._sem_poison
        self.nc.clear_and_free_semaphores(list(self.sems.allocated().values()))


def build_nc():
    nc = bacc.Bacc(
        "TRN2",
        target_bir_lowering=False,
        debug=False,
        enable_asserts=False,
        num_devices=N_CORES,
    )

    # packed per-(pair,half) byte planes; pair row ki holds
    # [hiA fp16 | hiB fp16 | lo pair-interleaved fp8] = 6*HALF bytes
    N_PAIRS = K_CHUNKS // 2
    xpk = nc.dram_tensor(
        "xpk", [N_PAIRS * 128, N_HALF * 6 * HALF], mybir.dt.uint8, kind="ExternalInput"
    ).ap()
    # pre-laid in SBUF order; first-4-chunk tiles separate so MMs start early
    wpka_d = nc.dram_tensor("wpka", [128, 8 * 128], F16, kind="ExternalInput").ap()
    wpkb_d = nc.dram_tensor(
        "wpkb", [128, (K_CHUNKS - 8) * 128], F16, kind="ExternalInput"
    ).ap()
    # DoubleRow weight pairs: per pair, [two=2, 64] fp8
    wl8a_d = nc.dram_tensor(
        "wl8a", [128, 4 * 2 * N_EXPERTS], F8, kind="ExternalInput"
    ).ap()
    wl8b_d = nc.dram_tensor(
        "wl8b", [128, (K_CHUNKS // 2 - 4) * 2 * N_EXPERTS], F8, kind="ExternalInput"
    ).ap()
    bias2 = nc.dram_tensor("bias2", [128, 1], F32, kind="ExternalInput").ap()
    identa = nc.dram_tensor("identa", [128, N_EXPERTS], F32, kind="ExternalInput").ap()

    w_out = nc.dram_tensor("topk_w", [T_CORE, TOP_K], F32, kind="ExternalOutput").ap()
    i_out = nc.dram_tensor("topk_i", [T_CORE, TOP_K], U32, kind="ExternalOutput").ap()
    s_out = nc.dram_tensor("stats", [128, 2], F32, kind="ExternalOutput").ap()

    with _LeanTileContext(nc) as tc:
        with (
            tc.tile_pool(name="const", bufs=1) as const_pool,
            tc.tile_pool(name="xt", bufs=12) as xt_pool,
            tc.tile_pool(name="lt", bufs=2) as lt_pool,
            tc.tile_pool(name="tok", bufs=2) as tok_pool,
            tc.tile_pool(name="outs", bufs=2) as out_pool,
            tc.tile_pool(name="psum_lt", bufs=2, space="PSUM") as psum_lt_pool,
            tc.tile_pool(name="psum_l", bufs=2, space="PSUM") as psum_l_pool,
            tc.tile_pool(name="psum_s", bufs=1, space="PSUM") as psum_s_pool,
        ):
            # --- constants ---
            # weights arrive host-pre-laid-out in SBUF order (contiguous
            # lines); first 4 chunks in their own tiles so MMs start early
            CS_A = 8
            wpk_a = const_pool.tile([128, CS_A * 128], F16)
            nc.sync.dma_start(wpk_a[:], wpka_d[:])
            wlo8_a = const_pool.tile([128, 4 * 2 * N_EXPERTS], F8)
            nc.sync.dma_start(wlo8_a[:], wl8a_d[:])
            wpk_b = const_pool.tile([128, (K_CHUNKS - CS_A) * 128], F16)
            wlo8_b = const_pool.tile(
                [128, (K_CHUNKS // 2 - 4) * 2 * N_EXPERTS], F8
            )

            def wsl_of(c):
                if c < CS_A:
                    return wpk_a[:, c * 128 : (c + 1) * 128]
                return wpk_b[:, (c - CS_A) * 128 : (c - CS_A + 1) * 128]

            def w8dr_of(p):
                # (128, 2, 64) fp8 DoubleRow weights for pair p
                w = 2 * N_EXPERTS
                if p < 4:
                    ap = wlo8_a[:, p * w : (p + 1) * w]
                else:
                    ap = wlo8_b[:, (p - 4) * w : (p - 4 + 1) * w]
                return ap.rearrange("p (two e) -> p two e", two=2)
            bias2_sb = const_pool.tile([128, 1], F32)
            identa_sb = const_pool.tile([128, N_EXPERTS], F32)
            ones_sb = const_pool.tile([128, 1], BF16)
            nc.vector.memset(ones_sb[:], 1.0)
            # stats accumulator in SBUF: col0 = sum_t score, col1 = counts
            acc_sb = const_pool.tile([128, 2], F32)
            nc.vector.memset(acc_sb[:], 0.0)

            half_state = {}

            def emit_post(h):
                ps, pslo = half_state.pop(h)
                for b in range(HALF // BLK):
                    tok0 = h * HALF + b * BLK
                    lt2_sb = lt_pool.tile(
                        [128, BLK], F32, name=f"lt2_{b}", tag=f"lt2_{b}"
                    )
                    nc.vector.tensor_scalar(
                        lt2_sb[:], ps[b][:], bias2_sb[:], None,
                        op0=mybir.AluOpType.add,
                    )
                    nc.vector.scalar_tensor_tensor(
                        out=lt2_sb[0:N_EXPERTS, :],
                        in0=pslo[b][:],
                        scalar=1.0 / (S_LO * S_W),
                        in1=lt2_sb[0:N_EXPERTS, :],
                        op0=mybir.AluOpType.mult,
                        op1=mybir.AluOpType.add,
                    )
                    # fused transpose + hi/lo-weight fold: lt2.T @ [I; I/S_W]
                    logits_blk = tok_pool.tile(
                        [128, SUB * N_EXPERTS], F32, name="logits_blk",
                        tag="logits_blk",
                    )
                    psum_l = psum_l_pool.tile(
                        [128, SUB * N_EXPERTS], F32, name="psum_l", tag="psum_l"
                    )
                    for t in range(SUB):
                        nc.tensor.matmul(
                            psum_l[:, t * N_EXPERTS : (t + 1) * N_EXPERTS],
                            lt2_sb[:, t * 128 : (t + 1) * 128],
                            identa_sb[:],
                            start=(t == 0),
                            stop=(t == SUB - 1),
                            skip_group_check=True,
                        )
                    nc.vector.tensor_copy(logits_blk[:], psum_l[:])

                    r = _topk_block(nc, (tok_pool, out_pool), logits_blk, SUB)

                    psum_s = psum_s_pool.tile([128, 2], F32, name="psum_s", tag="psum_s")
                    for t in range(SUB):
                        nc.tensor.matmul(
                            psum_s[:],
                            r["em"][:, t * 128 : (t + 1) * 128],
                            r["ro"][:, t * 2 : (t + 1) * 2],
                            start=(t == 0),
                            stop=(t == SUB - 1),
                            skip_group_check=True,
                        )
                    nc.vector.tensor_add(acc_sb[:], acc_sb[:], psum_s[:])

                    nc.sync.dma_start(
                        w_out[tok0 : tok0 + BLK, :].rearrange(
                            "(t p) k -> p t k", p=128
                        ),
                        r["w8"][:].rearrange("p (t k) -> p t k", t=SUB),
                    )
                    nc.sync.dma_start(
                        i_out[tok0 : tok0 + BLK, :].rearrange(
                            "(t p) k -> p t k", p=128
                        ),
                        r["idx"][:].rearrange("p (t k) -> p t k", t=SUB),
                    )

            for h in range(N_HALF):
                ps = [
                    psum_lt_pool.tile(
                        [128, BLK], F32, name=f"ps{b}", tag=f"ps{b}",
                        bufs=2 if b == 0 else 1,
                    )
                    for b in range(HALF // BLK)
                ]
                pslo = [
                    psum_lt_pool.tile(
                        [N_EXPERTS, BLK], F32, name=f"pslo{b}", tag=f"pslo{b}", bufs=1
                    )
                    for b in range(HALF // BLK)
                ]
                half_state[h] = (ps, pslo)
                for p in range(K_CHUNKS // 2):
                    xpk_t = xt_pool.tile([128, 6 * HALF], mybir.dt.uint8)
                    nc.sync.dma_start(
                        xpk_t[:],
                        xpk[p * 128 : (p + 1) * 128, h * 6 * HALF : (h + 1) * 6 * HALF],
                    )
                    xhia_t = xpk_t[:, 0 : 2 * HALF].bitcast(F16)
                    xhib_t = xpk_t[:, 2 * HALF : 4 * HALF].bitcast(F16)
                    xlo_t = (
                        xpk_t[:, 4 * HALF : 6 * HALF]
                        .bitcast(F8)
                        .rearrange("p (two n) -> p two n", two=2)
                    )
                    if h == 0 and p == 1:
                        nc.sync.dma_start(wpk_b[:], wpkb_d[:])
                        nc.sync.dma_start(wlo8_b[:], wl8b_d[:])
                        nc.sync.dma_start(bias2_sb[:], bias2[:])
                        nc.sync.dma_start(identa_sb[:], identa[:])
                    for b in range(HALF // BLK):
                        nc.tensor.matmul(
                            ps[b][:],
                            wsl_of(2 * p),
                            xhia_t[:, b * BLK : (b + 1) * BLK],
                            start=(p == 0),
                            stop=False,
                            skip_group_check=True,
                        )
                    for b in range(HALF // BLK):
                        nc.tensor.matmul(
                            ps[b][:],
                            wsl_of(2 * p + 1),
                            xhib_t[:, b * BLK : (b + 1) * BLK],
                            start=False,
                            stop=(p == K_CHUNKS // 2 - 1),
                            skip_group_check=True,
                        )
                    for b in range(HALF // BLK):
                        nc.tensor.matmul(
                            pslo[b][:],
                            w8dr_of(p),
                            xlo_t[:, :, b * BLK : (b + 1) * BLK],
                            start=(p == 0),
                            stop=(p == K_CHUNKS // 2 - 1),
                            skip_group_check=True,
                            perf_mode=mybir.MatmulPerfMode.DoubleRow,
                        )
                emit_post(h)

            nc.sync.dma_start(s_out[:], acc_sb[:])

    nc.compile()
    return nc


def _get_nc():
    global _CACHED_NC
    if _CACHED_NC is None:
        _CACHED_NC = build_nc()
    return _CACHED_NC


def kernel(hidden_states, router_weight, adaptive_bias, expert_quality_ema):
    global LAST_RESULTS
    import ml_dtypes

    f8 = ml_dtypes.float8_e4m3
    X = np.asarray(hidden_states, dtype=np.float32)
    W = np.asarray(router_weight, dtype=np.float32)
    ab = np.asarray(adaptive_bias, dtype=np.float32)
    ema = np.asarray(expert_quality_ema, dtype=np.float32)

    qb = ema / max(float(np.linalg.norm(ema)), 1e-12)
    bias2 = np.zeros((128, 1), dtype=np.float32)
    bias2[:N_EXPERTS, 0] = ab + qb
    wT = np.ascontiguousarray(W.T)  # (HIDDEN, 64) f32
    w_hi = wT.astype(np.float16)
    w_lo = ((wT - w_hi.astype(np.float32)) * S_W).astype(np.float16)
    wpk0 = np.concatenate([w_hi, w_lo], axis=1)  # (HIDDEN, 128) fp16
    wlo80 = (wT * S_W).astype(f8)  # (HIDDEN, 64) fp8

    def _sbuf_layout(warr, width, split):
        t = warr.reshape(-1, 128, width).transpose(1, 0, 2)
        a = np.ascontiguousarray(t[:, :split, :]).reshape(128, split * width)
        b = np.ascontiguousarray(t[:, split:, :]).reshape(128, -1)
        return a, b

    wpka, wpkb = _sbuf_layout(wpk0, 128, 8)
    # DoubleRow pair layout: (128, pair, two, 64)
    w8t = wlo80.reshape(K_CHUNKS // 2, 2, 128, N_EXPERTS).transpose(2, 0, 1, 3)
    wl8a = np.ascontiguousarray(w8t[:, :4]).reshape(128, -1)
    wl8b = np.ascontiguousarray(w8t[:, 4:]).reshape(128, -1)
    eye = np.eye(N_EXPERTS, dtype=np.float32)
    identa = np.vstack([eye, eye / S_W])
    xT = X.T  # (HIDDEN, N_TOKENS) view

    in_maps = []
    for c in range(N_CORES):
        shard = np.ascontiguousarray(xT[:, c * T_CORE : (c + 1) * T_CORE])
        s_hi = shard.astype(np.float16)
        s_lo = ((shard - s_hi.astype(np.float32)) * S_LO).astype(f8)
        # pair-major rows: row p*128+ki holds hiA/hiB fp16 + pair-interleaved fp8
        hi4 = s_hi.reshape(K_CHUNKS // 2, 2, 128, T_CORE)
        lo4 = s_lo.reshape(K_CHUNKS // 2, 2, 128, T_CORE)
        xpk = np.empty((K_CHUNKS // 2 * 128, N_HALF * 6 * HALF), dtype=np.uint8)
        xpk4 = xpk.reshape(K_CHUNKS // 2, 128, N_HALF, 6 * HALF)
        for h in range(N_HALF):
            tsl = slice(h * HALF, (h + 1) * HALF)
            xpk4[:, :, h, 0 : 2 * HALF] = (
                np.ascontiguousarray(hi4[:, 0, :, tsl]).view(np.uint8)
            )
            xpk4[:, :, h, 2 * HALF : 4 * HALF] = (
                np.ascontiguousarray(hi4[:, 1, :, tsl]).view(np.uint8)
            )
            xpk4[:, :, h, 4 * HALF : 5 * HALF] = (
                np.ascontiguousarray(lo4[:, 0, :, tsl]).view(np.uint8)
            )
            xpk4[:, :, h, 5 * HALF : 6 * HALF] = (
                np.ascontiguousarray(lo4[:, 1, :, tsl]).view(np.uint8)
            )
        in_maps.append(
            {
                "xpk": xpk,
                "wpka": wpka,
                "wpkb": wpkb,
                "wl8a": wl8a,
                "wl8b": wl8b,
                "bias2": bias2,
                "identa": identa,
            }
        )

    nc = _get_nc()
    trace = bool(os.environ.get("BASS_TRACE")) and _install_ntff_hook()
    try:
        res = run_bass_kernel_spmd(
            nc, in_maps, core_ids=list(range(N_CORES)), trace=trace
        )
    except Exception:
        if not trace:
            raise
        os.environ["BASS_NEVER_TRACE"] = "1"
        res = run_bass_kernel_spmd(
            nc, in_maps, core_ids=list(range(N_CORES)), trace=False
        )
    LAST_RESULTS = res

    topk_w = np.concatenate([r["topk_w"] for r in res.results], axis=0)
    topk_i = np.concatenate([r["topk_i"] for r in res.results], axis=0).astype(np.int32)
    stats = np.stack([r["stats"] for r in res.results]).sum(axis=0)  # (128, 2)
    mean_prob = stats[:N_EXPERTS, 0] / float(N_TOKENS)
    tokens_per_expert = stats[N_EXPERTS:, 1] / float(N_TOKENS * TOP_K)
    aux = np.float32(N_EXPERTS * np.sum(tokens_per_expert * mean_prob))
    return topk_w, topk_i, aux


# revision 40
# speedup vs baseline: 1.1042x; 1.0257x over previous
"""Trainium2 Bass kernel for AdaptiveRouter MoE routing.

reference:
  logits = hidden @ W^T + adaptive_bias + ema/||ema||  (N=16384, H=4096, E=64)
  scores = softmax(logits); topk_w, topk_i = top_k(scores, 8); topk_w /= sum
  aux = E * sum(tokens_per_expert * mean_prob)

Sharding: data-parallel over tokens across 8 cores (2048 tokens each);
router weight/bias replicated. Aux-loss partials (per-expert score sums,
per-expert top-8 counts) are computed per core and combined on the host
during the gather/unshard step.

Numerics / layout strategy (device is memory-bound; fp32 PE matmul runs
in slow 2-pass LOW_HIGH mode, and X^T is needed because the PE contracts
along partitions):
  - host transposes X and encodes each element in 3 bytes: fp16 hi plane
    plus fp8e4m3 residual plane scaled by 1024 (max logit err ~3.5e-5 ->
    0 of 131072 top-8 indices flip vs the fp32 reference on this data)
  - weights: [fp16(W^T) | (W^T - fp16(W^T))*64] packed 128-col stationary
    for the hi pass; fp8(W^T*64) in DoubleRow pair layout for the lo pass
  - the two planes are packed per (chunk-pair, token-half) into one uint8
    DRAM buffer so each (128, 6KB contiguous-line) tile arrives in ONE
    DMA (descriptor generation, ~0.7us per 128-line DMA, would otherwise
    pace the stream); on-chip views via AP.bitcast
  - hi pass: 2 fp16 MMs per pair into a (128, 512) PSUM block (rows 0-63
    accumulate W_hi terms, 64-127 W_lo terms); lo pass: 1 fp8 DoubleRow
    MM per pair (contracts both chunks at 2 elements/lane/cycle) into a
    (64, 512) PSUM
  - evacuation adds [bias; 0] (per-partition scalar) and folds the lo
    PSUM into rows 0-63 (scalar_tensor_tensor); a fp32 matmul against
    [I; I/64] then transposes to (token, expert) while summing the hi/lo
    weight rows
  - nc.vector.max / max_index give the top-8 (descending) per token;
    softmax needs no max subtraction (logits are O(6), fp32-safe);
    topk_w = exp(top8) / sum(exp(top8))
  - stats: one fused matmul per 128-token tile with lhsT = [exp(l) |
    top8-mask] bf16 and rhs = [1/denom | 1], contracting over the token
    partition dim; accumulated in PSUM/SBUF and summed on host
"""

import os

import numpy as np

import concourse.bass as bass
import concourse.mybir as mybir
from concourse import bacc
from concourse import tile
from concourse.bass_utils import run_bass_kernel_spmd

F32 = mybir.dt.float32
BF16 = mybir.dt.bfloat16
F16 = mybir.dt.float16
F8 = mybir.dt.float8e4
U32 = mybir.dt.uint32
S_LO = 1024.0  # host scale on the fp8 residual plane
S_W = 64.0  # host scale on the W-residual / fp8-W planes

N_TOKENS = 16384
HIDDEN = 4096
N_EXPERTS = 64
TOP_K = 8
N_CORES = 8
T_CORE = N_TOKENS // N_CORES  # 2048
K_CHUNKS = HIDDEN // 128  # 32
BLK = 512  # tokens per PSUM block
HALF = 1024  # tokens per outer iteration (2 PSUM blocks)
N_HALF = T_CORE // HALF  # 2
SUB = BLK // 128  # 4 sub-tiles of 128 tokens per block

_CACHED_NC = None
LAST_RESULTS = None


def _install_ntff_hook():
    """concourse's traced axon path imports antenv.axon_hooks, which this
    image lacks. Recreate it in sys.modules and register the ctypes-based
    NTFF profile hook from trn_agent_boot."""
    import sys
    import types

    if "antenv.axon_hooks" in sys.modules:
        return True
    try:
        import antenv
        from trn_agent_boot.trn_boot import _ntff_profile_via_ctypes

        mod = types.ModuleType("antenv.axon_hooks")
        mod._hook = _ntff_profile_via_ctypes("/opt/axon/libaxon_pjrt.so")

        def set_axon_ntff_profile_hook(h):
            mod._hook = h

        def get_axon_ntff_profile_hook():
            return mod._hook

        mod.set_axon_ntff_profile_hook = set_axon_ntff_profile_hook
        mod.get_axon_ntff_profile_hook = get_axon_ntff_profile_hook
        sys.modules["antenv.axon_hooks"] = mod
        antenv.axon_hooks = mod
        return True
    except Exception:
        return False


def _topk_block(nc, pools, logits_blk, nsub):
    """Top-8 + softmax + stats inputs over nsub 128-token sub-tiles.

    logits_blk: (128, nsub*64) f32 SBUF tile of logits in (token, expert)
    layout. Produces output tiles plus fused stats operands:
    em = [exp(l) | top8-mask] (128, nsub*128) bf16, ro = [recip | 1] pairs.
    """
    SUB = nsub
    tok_pool, out_pool = pools
    top8_blk = tok_pool.tile([128, SUB * TOP_K], F32)
    idx_blk = out_pool.tile([128, SUB * TOP_K], U32)
    em_blk = tok_pool.tile([128, SUB * 128], BF16)
    em3 = em_blk[:].rearrange("p (t c) -> p t c", t=SUB)
    for t in range(SUB):
        lsl = logits_blk[:, t * N_EXPERTS : (t + 1) * N_EXPERTS]
        t8 = top8_blk[:, t * TOP_K : (t + 1) * TOP_K]
        nc.vector.max(out=t8, in_=lsl)
        nc.vector.max_index(
            out=idx_blk[:, t * TOP_K : (t + 1) * TOP_K], in_max=t8, in_values=lsl
        )
    logits3 = logits_blk[:].rearrange("p (t e) -> p t e", t=SUB)
    # mask of top-8 positions (logits >= per-sub-tile 8th largest), bf16 0/1
    kth = top8_blk[:, TOP_K - 1 :: TOP_K].to_broadcast([128, SUB, N_EXPERTS])
    nc.vector.tensor_tensor(
        em3[:, :, N_EXPERTS:], logits3, kth, op=mybir.AluOpType.is_ge
    )
    # exp(l) for the aux-loss stats (bf16)
    nc.scalar.activation(
        em3[:, :, 0:N_EXPERTS], logits3, mybir.ActivationFunctionType.Exp
    )
    denom_blk = tok_pool.tile([128, SUB], F32)
    nc.vector.tensor_reduce(
        denom_blk[:],
        em3[:, :, 0:N_EXPERTS],
        axis=mybir.AxisListType.X,
        op=mybir.AluOpType.add,
    )
    recip_blk = tok_pool.tile([128, SUB], F32)
    nc.vector.reciprocal(recip_blk[:], denom_blk[:])
    ro_blk = tok_pool.tile([128, SUB * 2], BF16)
    nc.vector.memset(ro_blk[:], 1.0)
    nc.vector.tensor_copy(ro_blk[:, 0 : 2 * SUB : 2], recip_blk[:])

    # top-8 weights in f32
    e8_blk = tok_pool.tile([128, SUB * TOP_K], F32)
    nc.scalar.activation(e8_blk[:], top8_blk[:], mybir.ActivationFunctionType.Exp)
    s8_blk = tok_pool.tile([128, SUB], F32)
    nc.vector.tensor_reduce(
        s8_blk[:],
        e8_blk[:].rearrange("p (t k) -> p t k", t=SUB),
        axis=mybir.AxisListType.X,
        op=mybir.AluOpType.add,
    )
    r8_blk = tok_pool.tile([128, SUB], F32)
    nc.vector.reciprocal(r8_blk[:], s8_blk[:])
    w8_blk = out_pool.tile([128, SUB * TOP_K], F32)
    nc.vector.tensor_tensor(
        w8_blk[:].rearrange("p (t k) -> p t k", t=SUB),
        e8_blk[:].rearrange("p (t k) -> p t k", t=SUB),
        r8_blk[:].to_broadcast([128, SUB, TOP_K]),
        op=mybir.AluOpType.mult,
    )
    return dict(w8=w8_blk, idx=idx_blk, em=em_blk, ro=ro_blk)


class _LeanTileContext(tile.TileContext):
    # one end barrier instead of two: drain already waits for all sem
    # targets; the single barrier keeps sem-clears from racing pending
    # waiters, and nothing runs after the clears but engine halt.
    def _drain_and_barrier(self, tick_clock, wait_clock):
        from concourse.tile import ScopedClock

        drain_inst = self.nc.sync.drain()
        wait_clock.add_sem_waits(
            drain_inst.ins, ScopedClock({None: tick_clock.global_clock})
        )
        self.nc.all_engine_barrier()
        popped = self.nc._tile_sem_poison_stack.pop()
        assert popped is self._sem_poison
        self.nc.clear_and_free_semaphores(list(self.sems.allocated().values()))


def build_nc():
    nc = bacc.Bacc(
        "TRN2",
        target_bir_lowering=False,
        debug=False,
        enable_asserts=False,
        num_devices=N_CORES,
    )

    # packed per-(pair,half) byte planes; pair row ki holds
    # [hiA fp16 | hiB fp16 | lo pair-interleaved fp8] = 6*HALF bytes
    N_PAIRS = K_CHUNKS // 2
    xpk = nc.dram_tensor(
        "xpk", [N_PAIRS * 128, N_HALF * 6 * HALF], mybir.dt.uint8, kind="ExternalInput"
    ).ap()
    # pre-laid in SBUF order; first-4-chunk tiles separate so MMs start early
    wpka_d = nc.dram_tensor("wpka", [128, 8 * 128], F16, kind="ExternalInput").ap()
    wpkb_d = nc.dram_tensor(
        "wpkb", [128, (K_CHUNKS - 8) * 128], F16, kind="ExternalInput"
    ).ap()
    # DoubleRow weight pairs: per pair, [two=2, 64] fp8
    wl8a_d = nc.dram_tensor(
        "wl8a", [128, 4 * 2 * N_EXPERTS], F8, kind="ExternalInput"
    ).ap()
    wl8b_d = nc.dram_tensor(
        "wl8b", [128, (K_CHUNKS // 2 - 4) * 2 * N_EXPERTS], F8, kind="ExternalInput"
    ).ap()
    bias2 = nc.dram_tensor("bias2", [128, 1], F32, kind="ExternalInput").ap()
    identa = nc.dram_tensor("identa", [128, N_EXPERTS], F32, kind="ExternalInput").ap()

    w_out = nc.dram_tensor("topk_w", [T_CORE, TOP_K], F32, kind="ExternalOutput").ap()
    i_out = nc.dram_tensor("topk_i", [T_CORE, TOP_K], U32, kind="ExternalOutput").ap()
    s_out = nc.dram_tensor("stats", [128, 2], F32, kind="ExternalOutput").ap()

    with _LeanTileContext(nc) as tc:
        with (
            tc.tile_pool(name="const", bufs=1) as const_pool,
            tc.tile_pool(name="xt", bufs=12) as xt_pool,
            tc.tile_pool(name="lt", bufs=3) as lt_pool,
            tc.tile_pool(name="tok", bufs=3) as tok_pool,
            tc.tile_pool(name="outs", bufs=2) as out_pool,
            tc.tile_pool(name="psum_lt", bufs=2, space="PSUM") as psum_lt_pool,
            tc.tile_pool(name="psum_l", bufs=2, space="PSUM") as psum_l_pool,
            tc.tile_pool(name="psum_s", bufs=1, space="PSUM") as psum_s_pool,
        ):
            # --- constants ---
            # weights arrive host-pre-laid-out in SBUF order (contiguous
            # lines); first 4 chunks in their own tiles so MMs start early
            CS_A = 8
            wpk_a = const_pool.tile([128, CS_A * 128], F16)
            nc.sync.dma_start(wpk_a[:], wpka_d[:])
            wlo8_a = const_pool.tile([128, 4 * 2 * N_EXPERTS], F8)
            nc.sync.dma_start(wlo8_a[:], wl8a_d[:])
            wpk_b = const_pool.tile([128, (K_CHUNKS - CS_A) * 128], F16)
            wlo8_b = const_pool.tile(
                [128, (K_CHUNKS // 2 - 4) * 2 * N_EXPERTS], F8
            )

            def wsl_of(c):
                if c < CS_A:
                    return wpk_a[:, c * 128 : (c + 1) * 128]
                return wpk_b[:, (c - CS_A) * 128 : (c - CS_A + 1) * 128]

            def w8dr_of(p):
                # (128, 2, 64) fp8 DoubleRow weights for pair p
                w = 2 * N_EXPERTS
                if p < 4:
                    ap = wlo8_a[:, p * w : (p + 1) * w]
                else:
                    ap = wlo8_b[:, (p - 4) * w : (p - 4 + 1) * w]
                return ap.rearrange("p (two e) -> p two e", two=2)
            bias2_sb = const_pool.tile([128, 1], F32)
            identa_sb = const_pool.tile([128, N_EXPERTS], F32)
            ones_sb = const_pool.tile([128, 1], BF16)
            nc.vector.memset(ones_sb[:], 1.0)
            # stats accumulator in SBUF: col0 = sum_t score, col1 = counts
            acc_sb = const_pool.tile([128, 2], F32)
            nc.vector.memset(acc_sb[:], 0.0)

            half_state = {}

            def emit_post(h):
                ps, pslo = half_state.pop(h)
                for b in range(HALF // BLK):
                    tok0 = h * HALF + b * BLK
                    lt2_sb = lt_pool.tile(
                        [128, BLK], F32, name=f"lt2_{b}", tag=f"lt2_{b}"
                    )
                    nc.vector.tensor_scalar(
                        lt2_sb[:], ps[b][:], bias2_sb[:], None,
                        op0=mybir.AluOpType.add,
                    )
                    nc.vector.scalar_tensor_tensor(
                        out=lt2_sb[0:N_EXPERTS, :],
                        in0=pslo[b][:],
                        scalar=1.0 / (S_LO * S_W),
                        in1=lt2_sb[0:N_EXPERTS, :],
                        op0=mybir.AluOpType.mult,
                        op1=mybir.AluOpType.add,
                    )
                    # fused transpose + hi/lo-weight fold: lt2.T @ [I; I/S_W]
                    logits_blk = tok_pool.tile(
                        [128, SUB * N_EXPERTS], F32, name="logits_blk",
                        tag="logits_blk",
                    )
                    psum_l = psum_l_pool.tile(
                        [128, SUB * N_EXPERTS], F32, name="psum_l", tag="psum_l"
                    )
                    for t in range(SUB):
                        nc.tensor.matmul(
                            psum_l[:, t * N_EXPERTS : (t + 1) * N_EXPERTS],
                            lt2_sb[:, t * 128 : (t + 1) * 128],
                            identa_sb[:],
                            start=(t == 0),
                            stop=(t == SUB - 1),
                            skip_group_check=True,
                        )
                    nc.vector.tensor_copy(logits_blk[:], psum_l[:])

                    r = _topk_block(nc, (tok_pool, out_pool), logits_blk, SUB)

                    psum_s = psum_s_pool.tile([128, 2], F32, name="psum_s", tag="psum_s")
                    for t in range(SUB):
                        nc.tensor.matmul(
                            psum_s[:],
                            r["em"][:, t * 128 : (t + 1) * 128],
                            r["ro"][:, t * 2 : (t + 1) * 2],
                            start=(t == 0),
                            stop=(t == SUB - 1),
                            skip_group_check=True,
                        )
                    nc.vector.tensor_add(acc_sb[:], acc_sb[:], psum_s[:])

                    nc.sync.dma_start(
                        w_out[tok0 : tok0 + BLK, :].rearrange(
                            "(t p) k -> p t k", p=128
                        ),
                        r["w8"][:].rearrange("p (t k) -> p t k", t=SUB),
                    )
                    nc.sync.dma_start(
                        i_out[tok0 : tok0 + BLK, :].rearrange(
                            "(t p) k -> p t k", p=128
                        ),
                        r["idx"][:].rearrange("p (t k) -> p t k", t=SUB),
                    )

            for h in range(N_HALF):
                ps = [
                    psum_lt_pool.tile(
                        [128, BLK], F32, name=f"ps{b}", tag=f"ps{b}",
                        bufs=2 if b == 0 else 1,
                    )
                    for b in range(HALF // BLK)
                ]
                pslo = [
                    psum_lt_pool.tile(
                        [N_EXPERTS, BLK], F32, name=f"pslo{b}", tag=f"pslo{b}", bufs=1
                    )
                    for b in range(HALF // BLK)
                ]
                half_state[h] = (ps, pslo)
                for p in range(K_CHUNKS // 2):
                    xpk_t = xt_pool.tile([128, 6 * HALF], mybir.dt.uint8)
                    nc.sync.dma_start(
                        xpk_t[:],
                        xpk[p * 128 : (p + 1) * 128, h * 6 * HALF : (h + 1) * 6 * HALF],
                    )
                    xhia_t = xpk_t[:, 0 : 2 * HALF].bitcast(F16)
                    xhib_t = xpk_t[:, 2 * HALF : 4 * HALF].bitcast(F16)
                    xlo_t = (
                        xpk_t[:, 4 * HALF : 6 * HALF]
                        .bitcast(F8)
                        .rearrange("p (two n) -> p two n", two=2)
                    )
                    if h == 0 and p == 1:
                        nc.sync.dma_start(wpk_b[:], wpkb_d[:])
                        nc.sync.dma_start(wlo8_b[:], wl8b_d[:])
                        nc.sync.dma_start(bias2_sb[:], bias2[:])
                        nc.sync.dma_start(identa_sb[:], identa[:])
                    for b in range(HALF // BLK):
                        nc.tensor.matmul(
                            ps[b][:],
                            wsl_of(2 * p),
                            xhia_t[:, b * BLK : (b + 1) * BLK],
                            start=(p == 0),
                            stop=False,
                            skip_group_check=True,
                        )
                    for b in range(HALF // BLK):
                        nc.tensor.matmul(
                            ps[b][:],
                            wsl_of(2 * p + 1),
                            xhib_t[:, b * BLK : (b + 1) * BLK],
                            start=False,
                            stop=(p == K_CHUNKS // 2 - 1),
                            skip_group_check=True,
                        )
                    for b in range(HALF // BLK):
                        nc.tensor.matmul(
                            pslo[b][:],
                            w8dr_of(p),
                            xlo_t[:, :, b * BLK : (b + 1) * BLK],
                            start=(p == 0),
                            stop=(p == K_CHUNKS // 2 - 1),
                            skip_group_check=True,
                            perf_mode=mybir.MatmulPerfMode.DoubleRow,
                        )
                emit_post(h)

            nc.sync.dma_start(s_out[:], acc_sb[:])

    nc.compile()
    return nc


def _get_nc():
    global _CACHED_NC
    if _CACHED_NC is None:
        _CACHED_NC = build_nc()
    return _CACHED_NC


def kernel(hidden_states, router_weight, adaptive_bias, expert_quality_ema):
    global LAST_RESULTS
    import ml_dtypes

    f8 = ml_dtypes.float8_e4m3
    X = np.asarray(hidden_states, dtype=np.float32)
    W = np.asarray(router_weight, dtype=np.float32)
    ab = np.asarray(adaptive_bias, dtype=np.float32)
    ema = np.asarray(expert_quality_ema, dtype=np.float32)

    qb = ema / max(float(np.linalg.norm(ema)), 1e-12)
    bias2 = np.zeros((128, 1), dtype=np.float32)
    bias2[:N_EXPERTS, 0] = ab + qb
    wT = np.ascontiguousarray(W.T)  # (HIDDEN, 64) f32
    w_hi = wT.astype(np.float16)
    w_lo = ((wT - w_hi.astype(np.float32)) * S_W).astype(np.float16)
    wpk0 = np.concatenate([w_hi, w_lo], axis=1)  # (HIDDEN, 128) fp16
    wlo80 = (wT * S_W).astype(f8)  # (HIDDEN, 64) fp8

    def _sbuf_layout(warr, width, split):
        t = warr.reshape(-1, 128, width).transpose(1, 0, 2)
        a = np.ascontiguousarray(t[:, :split, :]).reshape(128, split * width)
        b = np.ascontiguousarray(t[:, split:, :]).reshape(128, -1)
        return a, b

    wpka, wpkb = _sbuf_layout(wpk0, 128, 8)
    # DoubleRow pair layout: (128, pair, two, 64)
    w8t = wlo80.reshape(K_CHUNKS // 2, 2, 128, N_EXPERTS).transpose(2, 0, 1, 3)
    wl8a = np.ascontiguousarray(w8t[:, :4]).reshape(128, -1)
    wl8b = np.ascontiguousarray(w8t[:, 4:]).reshape(128, -1)
    eye = np.eye(N_EXPERTS, dtype=np.float32)
    identa = np.vstack([eye, eye / S_W])
    xT = X.T  # (HIDDEN, N_TOKENS) view

    in_maps = []
    for c in range(N_CORES):
        shard = np.ascontiguousarray(xT[:, c * T_CORE : (c + 1) * T_CORE])
        s_hi = shard.astype(np.float16)
        s_lo = ((shard - s_hi.astype(np.float32)) * S_LO).astype(f8)
        # pair-major rows: row p*128+ki holds hiA/hiB fp16 + pair-interleaved fp8
        hi4 = s_hi.reshape(K_CHUNKS // 2, 2, 128, T_CORE)
        lo4 = s_lo.reshape(K_CHUNKS // 2, 2, 128, T_CORE)
        xpk = np.empty((K_CHUNKS // 2 * 128, N_HALF * 6 * HALF), dtype=np.uint8)
        xpk4 = xpk.reshape(K_CHUNKS // 2, 128, N_HALF, 6 * HALF)
        for h in range(N_HALF):
            tsl = slice(h * HALF, (h + 1) * HALF)
            xpk4[:, :, h, 0 : 2 * HALF] = (
                np.ascontiguousarray(hi4[:, 0, :, tsl]).view(np.uint8)
            )
            xpk4[:, :, h, 2 * HALF : 4 * HALF] = (
                np.ascontiguousarray(hi4[:, 1, :, tsl]).view(np.uint8)
            )
            xpk4[:, :, h, 4 * HALF : 5 * HALF] = (
                np.ascontiguousarray(lo4[:, 0, :, tsl]).view(np.uint8)
            )
            xpk4[:, :, h, 5 * HALF : 6 * HALF] = (
                np.ascontiguousarray(lo4[:, 1, :, tsl]).view(np.uint8)
            )
        in_maps.append(
            {
                "xpk": xpk,
                "wpka": wpka,
                "wpkb": wpkb,
                "wl8a": wl8a,
                "wl8b": wl8b,
                "bias2": bias2,
                "identa": identa,
            }
        )

    nc = _get_nc()
    trace = bool(os.environ.get("BASS_TRACE")) and _install_ntff_hook()
    try:
        res = run_bass_kernel_spmd(
            nc, in_maps, core_ids=list(range(N_CORES)), trace=trace
        )
    except Exception:
        if not trace:
            raise
        os.environ["BASS_NEVER_TRACE"] = "1"
        res = run_bass_kernel_spmd(
            nc, in_maps, core_ids=list(range(N_CORES)), trace=False
        )
    LAST_RESULTS = res

    topk_w = np.concatenate([r["topk_w"] for r in res.results], axis=0)
    topk_i = np.concatenate([r["topk_i"] for r in res.results], axis=0).astype(np.int32)
    stats = np.stack([r["stats"] for r in res.results]).sum(axis=0)  # (128, 2)
    mean_prob = stats[:N_EXPERTS, 0] / float(N_TOKENS)
    tokens_per_expert = stats[N_EXPERTS:, 1] / float(N_TOKENS * TOP_K)
    aux = np.float32(N_EXPERTS * np.sum(tokens_per_expert * mean_prob))
    return topk_w, topk_i, aux


# revision 41
# speedup vs baseline: 1.1182x; 1.0127x over previous
"""Trainium2 Bass kernel for AdaptiveRouter MoE routing.

reference:
  logits = hidden @ W^T + adaptive_bias + ema/||ema||  (N=16384, H=4096, E=64)
  scores = softmax(logits); topk_w, topk_i = top_k(scores, 8); topk_w /= sum
  aux = E * sum(tokens_per_expert * mean_prob)

Sharding: data-parallel over tokens across 8 cores (2048 tokens each);
router weight/bias replicated. Aux-loss partials (per-expert score sums,
per-expert top-8 counts) are computed per core and combined on the host
during the gather/unshard step.

Numerics / layout strategy (device is memory-bound; fp32 PE matmul runs
in slow 2-pass LOW_HIGH mode, and X^T is needed because the PE contracts
along partitions):
  - host transposes X and encodes each element in 3 bytes: fp16 hi plane
    plus fp8e4m3 residual plane scaled by 1024 (max logit err ~3.5e-5 ->
    0 of 131072 top-8 indices flip vs the fp32 reference on this data)
  - weights: [fp16(W^T) | (W^T - fp16(W^T))*64] packed 128-col stationary
    for the hi pass; fp8(W^T*64) in DoubleRow pair layout for the lo pass
  - the two planes are packed per (chunk-pair, token-half) into one uint8
    DRAM buffer so each (128, 6KB contiguous-line) tile arrives in ONE
    DMA (descriptor generation, ~0.7us per 128-line DMA, would otherwise
    pace the stream); on-chip views via AP.bitcast
  - hi pass: 2 fp16 MMs per pair into a (128, 512) PSUM block (rows 0-63
    accumulate W_hi terms, 64-127 W_lo terms); lo pass: 1 fp8 DoubleRow
    MM per pair (contracts both chunks at 2 elements/lane/cycle) into a
    (64, 512) PSUM
  - evacuation adds [bias; 0] (per-partition scalar) and folds the lo
    PSUM into rows 0-63 (scalar_tensor_tensor); a fp32 matmul against
    [I; I/64] then transposes to (token, expert) while summing the hi/lo
    weight rows
  - nc.vector.max / max_index give the top-8 (descending) per token;
    softmax needs no max subtraction (logits are O(6), fp32-safe);
    topk_w = exp(top8) / sum(exp(top8))
  - stats: one fused matmul per 128-token tile with lhsT = [exp(l) |
    top8-mask] bf16 and rhs = [1/denom | 1], contracting over the token
    partition dim; accumulated in PSUM/SBUF and summed on host
"""

import os

import numpy as np

import concourse.bass as bass
import concourse.mybir as mybir
from concourse import bacc
from concourse import tile
from concourse.bass_utils import run_bass_kernel_spmd

F32 = mybir.dt.float32
BF16 = mybir.dt.bfloat16
F16 = mybir.dt.float16
F8 = mybir.dt.float8e4
U32 = mybir.dt.uint32
S_LO = 1024.0  # host scale on the fp8 residual plane
S_W = 64.0  # host scale on the W-residual / fp8-W planes

N_TOKENS = 16384
HIDDEN = 4096
N_EXPERTS = 64
TOP_K = 8
N_CORES = 8
T_CORE = N_TOKENS // N_CORES  # 2048
K_CHUNKS = HIDDEN // 128  # 32
BLK = 512  # tokens per PSUM block
HALF = 1024  # tokens per outer iteration (2 PSUM blocks)
N_HALF = T_CORE // HALF  # 2
SUB = BLK // 128  # 4 sub-tiles of 128 tokens per block

_CACHED_NC = None
LAST_RESULTS = None


def _install_ntff_hook():
    """concourse's traced axon path imports antenv.axon_hooks, which this
    image lacks. Recreate it in sys.modules and register the ctypes-based
    NTFF profile hook from trn_agent_boot."""
    import sys
    import types

    if "antenv.axon_hooks" in sys.modules:
        return True
    try:
        import antenv
        from trn_agent_boot.trn_boot import _ntff_profile_via_ctypes

        mod = types.ModuleType("antenv.axon_hooks")
        mod._hook = _ntff_profile_via_ctypes("/opt/axon/libaxon_pjrt.so")

        def set_axon_ntff_profile_hook(h):
            mod._hook = h

        def get_axon_ntff_profile_hook():
            return mod._hook

        mod.set_axon_ntff_profile_hook = set_axon_ntff_profile_hook
        mod.get_axon_ntff_profile_hook = get_axon_ntff_profile_hook
        sys.modules["antenv.axon_hooks"] = mod
        antenv.axon_hooks = mod
        return True
    except Exception:
        return False


def _topk_block(nc, pools, logits_blk, nsub):
    """Top-8 + softmax + stats inputs over nsub 128-token sub-tiles.

    logits_blk: (128, nsub*64) f32 SBUF tile of logits in (token, expert)
    layout. Produces output tiles plus fused stats operands:
    em = [exp(l) | top8-mask] (128, nsub*128) bf16, ro = [recip | 1] pairs.
    """
    SUB = nsub
    tok_pool, out_pool = pools
    top8_blk = tok_pool.tile([128, SUB * TOP_K], F32)
    idx_blk = out_pool.tile([128, SUB * TOP_K], U32)
    em_blk = tok_pool.tile([128, SUB * 128], BF16)
    em3 = em_blk[:].rearrange("p (t c) -> p t c", t=SUB)
    for t in range(SUB):
        lsl = logits_blk[:, t * N_EXPERTS : (t + 1) * N_EXPERTS]
        t8 = top8_blk[:, t * TOP_K : (t + 1) * TOP_K]
        nc.vector.max(out=t8, in_=lsl)
        nc.vector.max_index(
            out=idx_blk[:, t * TOP_K : (t + 1) * TOP_K], in_max=t8, in_values=lsl
        )
    logits3 = logits_blk[:].rearrange("p (t e) -> p t e", t=SUB)
    # mask of top-8 positions (logits >= per-sub-tile 8th largest), bf16 0/1
    kth = top8_blk[:, TOP_K - 1 :: TOP_K].to_broadcast([128, SUB, N_EXPERTS])
    nc.vector.tensor_tensor(
        em3[:, :, N_EXPERTS:], logits3, kth, op=mybir.AluOpType.is_ge
    )
    # exp(l) for the aux-loss stats (bf16)
    nc.scalar.activation(
        em3[:, :, 0:N_EXPERTS], logits3, mybir.ActivationFunctionType.Exp
    )
    denom_blk = tok_pool.tile([128, SUB], F32)
    nc.vector.tensor_reduce(
        denom_blk[:],
        em3[:, :, 0:N_EXPERTS],
        axis=mybir.AxisListType.X,
        op=mybir.AluOpType.add,
    )
    recip_blk = tok_pool.tile([128, SUB], F32)
    nc.vector.reciprocal(recip_blk[:], denom_blk[:])
    ro_blk = tok_pool.tile([128, SUB * 2], BF16)
    nc.vector.memset(ro_blk[:], 1.0)
    nc.vector.tensor_copy(ro_blk[:, 0 : 2 * SUB : 2], recip_blk[:])

    # top-8 weights in f32
    e8_blk = tok_pool.tile([128, SUB * TOP_K], F32)
    nc.scalar.activation(e8_blk[:], top8_blk[:], mybir.ActivationFunctionType.Exp)
    s8_blk = tok_pool.tile([128, SUB], F32)
    nc.vector.tensor_reduce(
        s8_blk[:],
        e8_blk[:].rearrange("p (t k) -> p t k", t=SUB),
        axis=mybir.AxisListType.X,
        op=mybir.AluOpType.add,
    )
    r8_blk = tok_pool.tile([128, SUB], F32)
    nc.vector.reciprocal(r8_blk[:], s8_blk[:])
    w8_blk = out_pool.tile([128, SUB * TOP_K], F32)
    nc.vector.tensor_tensor(
        w8_blk[:].rearrange("p (t k) -> p t k", t=SUB),
        e8_blk[:].rearrange("p (t k) -> p t k", t=SUB),
        r8_blk[:].to_broadcast([128, SUB, TOP_K]),
        op=mybir.AluOpType.mult,
    )
    return dict(w8=w8_blk, idx=idx_blk, em=em_blk, ro=ro_blk)


class _LeanTileContext(tile.TileContext):
    # one end barrier instead of two: drain already waits for all sem
    # targets; the single barrier keeps sem-clears from racing pending
    # waiters, and nothing runs after the clears but engine halt.
    def _drain_and_barrier(self, tick_clock, wait_clock):
        from concourse.tile import ScopedClock

        drain_inst = self.nc.sync.drain()
        wait_clock.add_sem_waits(
            drain_inst.ins, ScopedClock({None: tick_clock.global_clock})
        )
        self.nc.all_engine_barrier()
        popped = self.nc._tile_sem_poison_stack.pop()
        assert popped is self._sem_poison
        self.nc.clear_and_free_semaphores(list(self.sems.allocated().values()))


def build_nc():
    nc = bacc.Bacc(
        "TRN2",
        target_bir_lowering=False,
        debug=False,
        enable_asserts=False,
        num_devices=N_CORES,
    )

    # packed per-(pair,half) byte planes; pair row ki holds
    # [hiA fp16 | hiB fp16 | lo pair-interleaved fp8] = 6*HALF bytes
    N_PAIRS = K_CHUNKS // 2
    xpk = nc.dram_tensor(
        "xpk", [N_PAIRS * 128, N_HALF * 6 * HALF], mybir.dt.uint8, kind="ExternalInput"
    ).ap()
    # pre-laid in SBUF order; first-4-chunk tiles separate so MMs start early
    wpka_d = nc.dram_tensor("wpka", [128, 8 * 128], F16, kind="ExternalInput").ap()
    wpkb_d = nc.dram_tensor(
        "wpkb", [128, (K_CHUNKS - 8) * 128], F16, kind="ExternalInput"
    ).ap()
    # DoubleRow weight pairs: per pair, [two=2, 64] fp8
    wl8a_d = nc.dram_tensor(
        "wl8a", [128, 4 * 2 * N_EXPERTS], F8, kind="ExternalInput"
    ).ap()
    wl8b_d = nc.dram_tensor(
        "wl8b", [128, (K_CHUNKS // 2 - 4) * 2 * N_EXPERTS], F8, kind="ExternalInput"
    ).ap()
    bias2 = nc.dram_tensor("bias2", [128, 1], F32, kind="ExternalInput").ap()
    identa = nc.dram_tensor("identa", [128, N_EXPERTS], F32, kind="ExternalInput").ap()

    w_out = nc.dram_tensor("topk_w", [T_CORE, TOP_K], F32, kind="ExternalOutput").ap()
    i_out = nc.dram_tensor("topk_i", [T_CORE, TOP_K], U32, kind="ExternalOutput").ap()
    s_out = nc.dram_tensor("stats", [128, 2], F32, kind="ExternalOutput").ap()

    with _LeanTileContext(nc) as tc:
        with (
            tc.tile_pool(name="const", bufs=1) as const_pool,
            tc.tile_pool(name="xt", bufs=12) as xt_pool,
            tc.tile_pool(name="lt", bufs=2) as lt_pool,
            tc.tile_pool(name="tok", bufs=2) as tok_pool,
            tc.tile_pool(name="outs", bufs=2) as out_pool,
            tc.tile_pool(name="psum_lt", bufs=2, space="PSUM") as psum_lt_pool,
            tc.tile_pool(name="psum_l", bufs=2, space="PSUM") as psum_l_pool,
            tc.tile_pool(name="psum_s", bufs=1, space="PSUM") as psum_s_pool,
        ):
            # --- constants ---
            # weights arrive host-pre-laid-out in SBUF order (contiguous
            # lines); first 4 chunks in their own tiles so MMs start early
            CS_A = 8
            wpk_a = const_pool.tile([128, CS_A * 128], F16)
            nc.sync.dma_start(wpk_a[:], wpka_d[:])
            wlo8_a = const_pool.tile([128, 4 * 2 * N_EXPERTS], F8)
            nc.sync.dma_start(wlo8_a[:], wl8a_d[:])
            wpk_b = const_pool.tile([128, (K_CHUNKS - CS_A) * 128], F16)
            wlo8_b = const_pool.tile(
                [128, (K_CHUNKS // 2 - 4) * 2 * N_EXPERTS], F8
            )

            def wsl_of(c):
                if c < CS_A:
                    return wpk_a[:, c * 128 : (c + 1) * 128]
                return wpk_b[:, (c - CS_A) * 128 : (c - CS_A + 1) * 128]

            def w8dr_of(p):
                # (128, 2, 64) fp8 DoubleRow weights for pair p
                w = 2 * N_EXPERTS
                if p < 4:
                    ap = wlo8_a[:, p * w : (p + 1) * w]
                else:
                    ap = wlo8_b[:, (p - 4) * w : (p - 4 + 1) * w]
                return ap.rearrange("p (two e) -> p two e", two=2)
            bias2_sb = const_pool.tile([128, 1], F32)
            identa_sb = const_pool.tile([128, N_EXPERTS], F32)
            ones_sb = const_pool.tile([128, 1], BF16)
            nc.vector.memset(ones_sb[:], 1.0)
            # stats accumulator in SBUF: col0 = sum_t score, col1 = counts
            acc_sb = const_pool.tile([128, 2], F32)
            nc.vector.memset(acc_sb[:], 0.0)

            half_state = {}

            def emit_post(h):
                ps, pslo = half_state.pop(h)
                for b in range(HALF // BLK):
                    tok0 = h * HALF + b * BLK
                    lt2_sb = lt_pool.tile(
                        [128, BLK], F32, name=f"lt2_{b}", tag=f"lt2_{b}"
                    )
                    nc.vector.tensor_scalar(
                        lt2_sb[:], ps[b][:], bias2_sb[:], None,
                        op0=mybir.AluOpType.add,
                    )
                    nc.vector.scalar_tensor_tensor(
                        out=lt2_sb[0:N_EXPERTS, :],
                        in0=pslo[b][:],
                        scalar=1.0 / (S_LO * S_W),
                        in1=lt2_sb[0:N_EXPERTS, :],
                        op0=mybir.AluOpType.mult,
                        op1=mybir.AluOpType.add,
                    )
                    # fused transpose + hi/lo-weight fold: lt2.T @ [I; I/S_W]
                    logits_blk = tok_pool.tile(
                        [128, SUB * N_EXPERTS], F32, name="logits_blk",
                        tag="logits_blk",
                    )
                    psum_l = psum_l_pool.tile(
                        [128, SUB * N_EXPERTS], F32, name="psum_l", tag="psum_l"
                    )
                    for t in range(SUB):
                        nc.tensor.matmul(
                            psum_l[:, t * N_EXPERTS : (t + 1) * N_EXPERTS],
                            lt2_sb[:, t * 128 : (t + 1) * 128],
                            identa_sb[:],
                            start=(t == 0),
                            stop=(t == SUB - 1),
                            skip_group_check=True,
                        )
                    nc.vector.tensor_copy(logits_blk[:], psum_l[:])

                    r = _topk_block(nc, (tok_pool, out_pool), logits_blk, SUB)

                    psum_s = psum_s_pool.tile([128, 2], F32, name="psum_s", tag="psum_s")
                    for t in range(SUB):
                        nc.tensor.matmul(
                            psum_s[:],
                            r["em"][:, t * 128 : (t + 1) * 128],
                            r["ro"][:, t * 2 : (t + 1) * 2],
                            start=(t == 0),
                            stop=(t == SUB - 1),
                            skip_group_check=True,
                        )
                    nc.vector.tensor_add(acc_sb[:], acc_sb[:], psum_s[:])

                    nc.sync.dma_start(
                        w_out[tok0 : tok0 + BLK, :].rearrange(
                            "(t p) k -> p t k", p=128
                        ),
                        r["w8"][:].rearrange("p (t k) -> p t k", t=SUB),
                    )
                    nc.sync.dma_start(
                        i_out[tok0 : tok0 + BLK, :].rearrange(
                            "(t p) k -> p t k", p=128
                        ),
                        r["idx"][:].rearrange("p (t k) -> p t k", t=SUB),
                    )

            for h in range(N_HALF):
                ps = [
                    psum_lt_pool.tile(
                        [128, BLK], F32, name=f"ps{b}", tag=f"ps{b}",
                        bufs=2 if b == 0 else 1,
                    )
                    for b in range(HALF // BLK)
                ]
                pslo = [
                    psum_lt_pool.tile(
                        [N_EXPERTS, BLK], F32, name=f"pslo{b}", tag=f"pslo{b}", bufs=1
                    )
                    for b in range(HALF // BLK)
                ]
                half_state[h] = (ps, pslo)
                for p in range(K_CHUNKS // 2):
                    xpk_t = xt_pool.tile([128, 6 * HALF], mybir.dt.uint8)
                    nc.sync.dma_start(
                        xpk_t[:],
                        xpk[p * 128 : (p + 1) * 128, h * 6 * HALF : (h + 1) * 6 * HALF],
                    )
                    xhia_t = xpk_t[:, 0 : 2 * HALF].bitcast(F16)
                    xhib_t = xpk_t[:, 2 * HALF : 4 * HALF].bitcast(F16)
                    xlo_t = (
                        xpk_t[:, 4 * HALF : 6 * HALF]
                        .bitcast(F8)
                        .rearrange("p (two n) -> p two n", two=2)
                    )
                    if h == 0 and p == 1:
                        nc.sync.dma_start(wpk_b[:], wpkb_d[:])
                        nc.sync.dma_start(wlo8_b[:], wl8b_d[:])
                        nc.sync.dma_start(bias2_sb[:], bias2[:])
                        nc.sync.dma_start(identa_sb[:], identa[:])
                    for b in range(HALF // BLK):
                        nc.tensor.matmul(
                            ps[b][:],
                            wsl_of(2 * p),
                            xhia_t[:, b * BLK : (b + 1) * BLK],
                            start=(p == 0),
                            stop=False,
                            skip_group_check=True,
                        )
                    for b in range(HALF // BLK):
                        nc.tensor.matmul(
                            ps[b][:],
                            wsl_of(2 * p + 1),
                            xhib_t[:, b * BLK : (b + 1) * BLK],
                            start=False,
                            stop=(p == K_CHUNKS // 2 - 1),
                            skip_group_check=True,
                        )
                    for b in range(HALF // BLK):
                        nc.tensor.matmul(
                            pslo[b][:],
                            w8dr_of(p),
                            xlo_t[:, :, b * BLK : (b + 1) * BLK],
                            start=(p == 0),
                            stop=(p == K_CHUNKS // 2 - 1),
                            skip_group_check=True,
                            perf_mode=mybir.MatmulPerfMode.DoubleRow,
                        )
                emit_post(h)

            nc.sync.dma_start(s_out[:], acc_sb[:])

    nc.compile()
    return nc


def _get_nc():
    global _CACHED_NC
    if _CACHED_NC is None:
        _CACHED_NC = build_nc()
    return _CACHED_NC


def kernel(hidden_states, router_weight, adaptive_bias, expert_quality_ema):
    global LAST_RESULTS
    import ml_dtypes

    f8 = ml_dtypes.float8_e4m3
    X = np.asarray(hidden_states, dtype=np.float32)
    W = np.asarray(router_weight, dtype=np.float32)
    ab = np.asarray(adaptive_bias, dtype=np.float32)
    ema = np.asarray(expert_quality_ema, dtype=np.float32)

    qb = ema / max(float(np.linalg.norm(ema)), 1e-12)
    bias2 = np.zeros((128, 1), dtype=np.float32)
    bias2[:N_EXPERTS, 0] = ab + qb
    wT = np.ascontiguousarray(W.T)  # (HIDDEN, 64) f32
    w_hi = wT.astype(np.float16)
    w_lo = ((wT - w_hi.astype(np.float32)) * S_W).astype(np.float16)
    wpk0 = np.concatenate([w_hi, w_lo], axis=1)  # (HIDDEN, 128) fp16
    wlo80 = (wT * S_W).astype(f8)  # (HIDDEN, 64) fp8

    def _sbuf_layout(warr, width, split):
        t = warr.reshape(-1, 128, width).transpose(1, 0, 2)
        a = np.ascontiguousarray(t[:, :split, :]).reshape(128, split * width)
        b = np.ascontiguousarray(t[:, split:, :]).reshape(128, -1)
        return a, b

    wpka, wpkb = _sbuf_layout(wpk0, 128, 8)
    # DoubleRow pair layout: (128, pair, two, 64)
    w8t = wlo80.reshape(K_CHUNKS // 2, 2, 128, N_EXPERTS).transpose(2, 0, 1, 3)
    wl8a = np.ascontiguousarray(w8t[:, :4]).reshape(128, -1)
    wl8b = np.ascontiguousarray(w8t[:, 4:]).reshape(128, -1)
    eye = np.eye(N_EXPERTS, dtype=np.float32)
    identa = np.vstack([eye, eye / S_W])
    xT = X.T  # (HIDDEN, N_TOKENS) view

    in_maps = []
    for c in range(N_CORES):
        shard = np.ascontiguousarray(xT[:, c * T_CORE : (c + 1) * T_CORE])
        s_hi = shard.astype(np.float16)
        s_lo = ((shard - s_hi.astype(np.float32)) * S_LO).astype(f8)
        # pair-major rows: row p*128+ki holds hiA/hiB fp16 + pair-interleaved fp8
        hi4 = s_hi.reshape(K_CHUNKS // 2, 2, 128, T_CORE)
        lo4 = s_lo.reshape(K_CHUNKS // 2, 2, 128, T_CORE)
        xpk = np.empty((K_CHUNKS // 2 * 128, N_HALF * 6 * HALF), dtype=np.uint8)
        xpk4 = xpk.reshape(K_CHUNKS // 2, 128, N_HALF, 6 * HALF)
        for h in range(N_HALF):
            tsl = slice(h * HALF, (h + 1) * HALF)
            xpk4[:, :, h, 0 : 2 * HALF] = (
                np.ascontiguousarray(hi4[:, 0, :, tsl]).view(np.uint8)
            )
            xpk4[:, :, h, 2 * HALF : 4 * HALF] = (
                np.ascontiguousarray(hi4[:, 1, :, tsl]).view(np.uint8)
            )
            xpk4[:, :, h, 4 * HALF : 5 * HALF] = (
                np.ascontiguousarray(lo4[:, 0, :, tsl]).view(np.uint8)
            )
            xpk4[:, :, h, 5 * HALF : 6 * HALF] = (
                np.ascontiguousarray(lo4[:, 1, :, tsl]).view(np.uint8)
            )
        in_maps.append(
            {
                "xpk": xpk,
                "wpka": wpka,
                "wpkb": wpkb,
                "wl8a": wl8a,
                "wl8b": wl8b,
                "bias2": bias2,
                "identa": identa,
            }
        )

    nc = _get_nc()
    trace = bool(os.environ.get("BASS_TRACE")) and _install_ntff_hook()
    try:
        res = run_bass_kernel_spmd(
            nc, in_maps, core_ids=list(range(N_CORES)), trace=trace
        )
    except Exception:
        if not trace:
            raise
        os.environ["BASS_NEVER_TRACE"] = "1"
        res = run_bass_kernel_spmd(
            nc, in_maps, core_ids=list(range(N_CORES)), trace=False
        )
    LAST_RESULTS = res

    topk_w = np.concatenate([r["topk_w"] for r in res.results], axis=0)
    topk_i = np.concatenate([r["topk_i"] for r in res.results], axis=0).astype(np.int32)
    stats = np.stack([r["stats"] for r in res.results]).sum(axis=0)  # (128, 2)
    mean_prob = stats[:N_EXPERTS, 0] / float(N_TOKENS)
    tokens_per_expert = stats[N_EXPERTS:, 1] / float(N_TOKENS * TOP_K)
    aux = np.float32(N_EXPERTS * np.sum(tokens_per_expert * mean_prob))
    return topk_w, topk_i, aux
